# revision 1
# baseline (speedup 1.0000x reference)
"""Trainium2 Bass kernel for nn_Dynamics (GNN message passing).

Data-parallel over batch n=1024 across 8 NeuronCores (128 rows each).
All activations on-chip are channel-major: (channels, batch*obj) so every
Linear is a single PE matmul with the stored (fan_in, fan_out) weight as lhsT.

The all-pairs first layer is built by PSUM accumulation of three matmuls per
512-pair block:
    pair1 = Wi.T @ x_i  +  Wj.T @ x_j  +  Wd.T @ dist
where x_i / x_j are step-0 broadcast views of the (32, n*o) state tensor and
dist rows are streamed from a repacked (1, 8192) per-chunk buffer.
rel and att towers are fused along the output-channel (M) dimension.
"""

import numpy as np
from contextlib import ExitStack

import concourse.bass as bass
import concourse.mybir as mybir
import concourse.tile as tile
from concourse import bacc

F32 = mybir.dt.float32
F32R = mybir.dt.float32r
AF = mybir.ActivationFunctionType
ALU = mybir.AluOpType
AX = mybir.AxisListType

N = 1024
NOBJ = 32
CL = 32
NCORES = 8
NP = N // NCORES          # 128 batch rows per core
NO = NP * NOBJ            # 4096 objects per core
NPAIR = NP * NOBJ * NOBJ  # 131072 pairs per core
BLK = 512                 # pairs per PSUM bank (fp32)
NCHUNK = 16               # n-chunks of 8 batch rows
CHUNK_N = NP // NCHUNK    # 8


def _build_program():
    nc = bacc.Bacc("TRN2", target_bir_lowering=False, debug=False)

    def din(name, shape):
        return nc.dram_tensor(name, list(shape), F32, kind="ExternalInput").ap()

    s3d = din("s3d", (NP, NOBJ, 16))
    w_s2 = din("w_s2", (128, 32))      # enc weights replicated on 4 strips
    b_s2 = din("b_s2", (32, 1))
    wi_cat = din("wi_cat", (32, 128))
    wj_cat = din("wj_cat", (32, 128))
    wd_cat = din("wd_cat", (1, 128))
    b_l1 = din("b_l1", (128, 1))
    w1_cat = din("w1_cat", (128, 64))  # blockdiag(rel_w1, att_w1)
    b_l2c = din("b_l2c", (64, 1))      # [rel_b1; att_b1]
    w2_cat = din("w2_cat", (64, 33))   # blockdiag(rel_w2, att_w2)
    b3_relc = din("b3_relc", (33, 1))  # [rel_b2; 0] (bias folded in p3 evac)
    b3_att = din("b3_att", (32, 1))    # att_b2 replicated (exp bias)
    diagneg = din("diagneg", (33, 2 * BLK))  # row 32: -1e30 on diag cols
    ones2b = din("ones2b", (33, 32))   # all-ones at row 32
    negrow = din("negrow", (33, 32))   # row 32 = -1e30 (diag lhsT)
    w_self0 = din("w_self0", (32, 32))
    b_self0 = din("b_self0", (32, 1))
    w_self1 = din("w_self1", (32, 32))
    b_self1 = din("b_self1", (32, 1))
    w_aff0 = din("w_aff0", (32, 32))
    b_aff0 = din("b_aff0", (32, 1))
    w_aff1 = din("w_aff1", (32, 32))
    b_aff1 = din("b_aff1", (32, 1))
    w_aff2 = din("w_aff2", (32, 32))
    b_aff2 = din("b_aff2", (32, 1))
    w_out0 = din("w_out0", (64, 32))
    b_out0 = din("b_out0", (32, 1))
    w_out1 = din("w_out1", (32, 32))
    b_out1 = din("b_out1", (32, 1))
    iden = din("iden", (128, 128))

    out_d = nc.dram_tensor("out", [NP, NOBJ, CL], F32, kind="ExternalOutput").ap()

    with tile.TileContext(nc) as tc, ExitStack() as ctx:
        const = ctx.enter_context(tc.tile_pool(name="const", bufs=1))

        def load_const(ap_in, shape=None, f32r=False):
            nm = f"t_{ap_in.name}"
            t = const.tile(list(shape), F32, name=nm, tag=nm)
            nc.gpsimd.dma_start(out=t, in_=ap_in)
            if f32r:
                tr = const.tile(list(shape), F32R, name=nm + "_r", tag=nm + "_r")
                nc.vector.tensor_copy(tr, t)
                return tr
            return t

        # --- constants to SBUF ---
        t_ws2 = load_const(w_s2, (128, 32))
        t_bs2 = load_const(b_s2, (32, 1))
        t_wi = load_const(wi_cat, f32r=True, shape=(32, 128))
        t_wj = load_const(wj_cat, f32r=True, shape=(32, 128))
        t_wd = load_const(wd_cat, f32r=True, shape=(1, 128))
        t_bl1 = load_const(b_l1, (128, 1))
        t_w1c = load_const(w1_cat, f32r=True, shape=(128, 64))
        t_bl2c = load_const(b_l2c, (64, 1))
        t_w2c = load_const(w2_cat, f32r=True, shape=(64, 33))
        t_b3r = load_const(b3_relc, (33, 1))
        t_b3a = load_const(b3_att, (32, 1))
        t_diag = load_const(diagneg, f32r=True, shape=(33, 2 * BLK))
        t_o2b = load_const(ones2b, f32r=True, shape=(33, 32))
        t_negr = load_const(negrow, f32r=True, shape=(33, 32))
        t_wself0 = load_const(w_self0, f32r=True, shape=(32, 32))
        t_bself0 = load_const(b_self0, (32, 1))
        t_wself1 = load_const(w_self1, f32r=True, shape=(32, 32))
        t_bself1 = load_const(b_self1, (32, 1))
        t_waff0 = load_const(w_aff0, f32r=True, shape=(32, 32))
        t_baff0 = load_const(b_aff0, (32, 1))
        t_waff1 = load_const(w_aff1, f32r=True, shape=(32, 32))
        t_baff1 = load_const(b_aff1, (32, 1))
        t_waff2 = load_const(w_aff2, f32r=True, shape=(32, 32))
        t_baff2 = load_const(b_aff2, (32, 1))
        t_wout0 = load_const(w_out0, f32r=True, shape=(64, 32))
        t_bout0 = load_const(b_out0, (32, 1))
        t_wout1 = load_const(w_out1, f32r=True, shape=(32, 32))
        t_bout1 = load_const(b_out1, (32, 1))
        t_iden = load_const(iden, (128, 128))

        # --- state loads: (n, o, c) padded to c=32 so transposes land 32-aligned ---
        s_pad = const.tile([NP, NOBJ, 32], F32)
        nc.gpsimd.memset(s_pad, 0.0)
        nc.gpsimd.dma_start(out=s_pad[:, :, 0:16], in_=s3d)
        pre_cm = tc.tile_pool(name="pre", bufs=1)
        pre = pre_cm.__enter__()
        pre_ps_cm = tc.tile_pool(name="pre_ps", bufs=2, space="PSUM")
        pre_ps = pre_ps_cm.__enter__()

        # --- s2 = concat(pos, enc[2:]) channel-major, via PE transposes ---
        # transpose chunk k: (n, 4 o's x 32c) -> ((o_l,c), n); then per o one
        # K=16 matmul from strip 32*o_l gives s2[:, (n, o)] columns (stride 32)
        s2 = const.tile([32, NO], F32R)
        s_flat = s_pad.rearrange("p a b -> p (a b)")     # (128, 1024)
        s2_3d = s2.rearrange("p (n o) -> p n o", o=NOBJ)
        for k in range(8):
            pst = pre_ps.tile([128, 128], F32, tag="ps_tr", name="pst")
            nc.tensor.transpose(pst, s_flat[:, 128 * k:128 * (k + 1)], t_iden)
            xt = pre.tile([128, 128], F32, tag="xt", bufs=2, name="xt")
            nc.vector.tensor_copy(xt, pst)
            for ol in range(4):
                o = 4 * k + ol
                ps = pre_ps.tile([32, 128], F32, tag="ps_enc", name="ps_enc")
                nc.tensor.matmul(ps, t_ws2[32 * ol:32 * ol + 16],
                                 xt[32 * ol:32 * ol + 16],
                                 start=True, stop=True,
                                 tile_position=(32 * ol, 0))
                nc.scalar.activation(s2_3d[:, :, o], ps, AF.Identity,
                                     bias=t_bs2)

        # --- pairwise squared distances, n on partitions ---
        pos = s_pad[:, :, 0:2]
        diff = pre.tile([NP, NOBJ, NOBJ, 2], F32)
        nc.vector.tensor_tensor(
            out=diff,
            in0=pos.unsqueeze(2).broadcast_to([NP, NOBJ, NOBJ, 2]),
            in1=pos.unsqueeze(1).broadcast_to([NP, NOBJ, NOBJ, 2]),
            op=ALU.subtract,
        )
        sq = pre.tile([NP, NOBJ, NOBJ, 2], F32)
        nc.vector.tensor_tensor(out=sq, in0=diff, in1=diff, op=ALU.mult)
        dist_t = const.tile([NP, NOBJ, NOBJ], F32)
        nc.vector.tensor_reduce(dist_t, sq, AX.X, ALU.add)
        dist_tr = const.tile([NP, NOBJ, NOBJ], F32R)
        nc.vector.tensor_copy(dist_tr, dist_t)

        # --- self-dynamics (tiny) ---
        h1 = pre.tile([32, NO], F32R)
        for blk in range(NO // BLK):
            ps = pre_ps.tile([32, BLK], F32, tag="ps_small")
            nc.tensor.matmul(ps, t_wself0.bitcast(F32R), s2[:, blk * BLK:(blk + 1) * BLK].bitcast(F32R),
                             start=True, stop=True)
            nc.scalar.activation(h1[:, blk * BLK:(blk + 1) * BLK], ps,
                                 AF.Relu, bias=t_bself0)
        selfd = const.tile([32, NO], F32)
        for blk in range(NO // BLK):
            ps = pre_ps.tile([32, BLK], F32, tag="ps_small")
            nc.tensor.matmul(ps, t_wself1.bitcast(F32R), h1[:, blk * BLK:(blk + 1) * BLK].bitcast(F32R),
                             start=True, stop=True)
            nc.vector.scalar_tensor_tensor(
                out=selfd[:, blk * BLK:(blk + 1) * BLK], in0=ps,
                scalar=t_bself1, in1=h1[:, blk * BLK:(blk + 1) * BLK].bitcast(F32),
                op0=ALU.add, op1=ALU.add)

        pre_ps_cm.__exit__(None, None, None)
        pre_cm.__exit__(None, None, None)

        # --- pair loop: one 512-pair block (nh, ih) per iteration ---
        # All matmuls are float32r with PSUM dst at partition 0 (HW requires it).
        # L2/L3 use block-diagonal fused weights so rel+att run in one matmul.
        rel_dyn_cm = const.tile([32, NO], F32)
        work_cm = tc.tile_pool(name="work", bufs=4)
        work = work_cm.__enter__()
        distp_cm = tc.tile_pool(name="distp", bufs=2)
        distp = distp_cm.__enter__()
        psA_cm = tc.tile_pool(name="psA", bufs=2, space="PSUM")
        psA = psA_cm.__enter__()
        psB_cm = tc.tile_pool(name="psB", bufs=2, space="PSUM")
        psB = psB_cm.__enter__()
        for c in range(NCHUNK):
            dist_f = distp.tile([1, CHUNK_N * 1024], F32R, tag="dist_f")
            nc.sync.dma_start(
                out=dist_f,
                in_=dist_tr[8 * c: 8 * c + 8].rearrange("p a b -> p (a b)"),
            )
            for nl in range(CHUNK_N):
                nh = c * CHUNK_N + nl  # batch row within core
                prodn = work.tile([32, 2, 16, NOBJ], F32, tag="prodn")
                for ih in range(2):
                    # L1: pair1 = Wi.T x_i + Wj.T x_j + Wd.T dist  (128, 512)
                    p1 = psA.tile([128, 16, NOBJ], F32, tag="p1")
                    xi = s2[:, nh * 32 + ih * 16: nh * 32 + ih * 16 + 16]
                    xi = xi.unsqueeze(2).broadcast_to([32, 16, NOBJ])
                    xj = s2[:, nh * 32: nh * 32 + 32]
                    xj = xj.unsqueeze(1).broadcast_to([32, 16, NOBJ])
                    drow = dist_f[0:1,
                                  nl * 1024 + ih * BLK: nl * 1024 + (ih + 1) * BLK]
                    nc.tensor.matmul(p1, t_wi, xi.bitcast(F32R), start=True, stop=False)
                    nc.tensor.matmul(p1, t_wj, xj.bitcast(F32R), start=False, stop=False)
                    nc.tensor.matmul(
                        p1.rearrange("p a b -> p (a b)"), t_wd, drow,
                        start=False, stop=True)
                    f1 = p1.rearrange("p a b -> p (a b)")
                    pair1 = work.tile([128, BLK], F32R, tag="pair1")
                    nc.scalar.activation(pair1, f1, AF.Relu, bias=t_bl1)
                    # L2: one K=128 block-diag matmul -> [rel2(32); att2(32)]
                    p2 = psB.tile([64, BLK], F32, tag="p2")
                    nc.tensor.matmul(p2, t_w1c, pair1, start=True, stop=True)
                    pair2 = work.tile([64, BLK], F32R, tag="pair2")
                    nc.vector.tensor_scalar(
                        out=pair2, in0=p2, scalar1=t_bl2c, scalar2=0.0,
                        op0=ALU.add, op1=ALU.max)
                    # L3 (+folded residual, +bias in evac): [relf(32); att3(1)]
                    p3 = psB.tile([33, BLK], F32, tag="p3")
                    nc.tensor.matmul(p3, t_w2c, pair2, start=True, stop=True)
                    attE = work.tile([33, BLK], F32R, tag="attE")
                    nc.vector.tensor_scalar(out=attE, in0=p3, scalar1=t_b3r,
                                            scalar2=None, op0=ALU.add)
                    pwp = psB.tile([32, BLK], F32, tag="pwp", bufs=1)
                    nc.tensor.matmul(pwp, t_o2b[32:33], attE[32:33],
                                     start=True, stop=False, tile_position=(32, 0))
                    nc.tensor.matmul(pwp, t_negr[32:33],
                                     t_diag[32:33, ih * BLK:(ih + 1) * BLK],
                                     start=False, stop=True, tile_position=(32, 0))
                    attW = work.tile([32, BLK], F32, tag="attW")
                    nc.scalar.activation(attW, pwp, AF.Exp, bias=t_b3a)
                    # weighted product; reduced once per batch row below
                    nc.gpsimd.tensor_tensor(
                        out=prodn[:, ih],
                        in0=attE[0:32].bitcast(F32).rearrange("p (a b) -> p a b", b=NOBJ),
                        in1=attW.rearrange("p (a b) -> p a b", b=NOBJ),
                        op=ALU.mult)
                prodh = work.tile([32, 2, 16, NOBJ // 2], F32, tag="prodh")
                nc.gpsimd.tensor_tensor(
                    out=prodh,
                    in0=prodn.rearrange("p k a (b t) -> p k a b t", t=2)[:, :, :, :, 0],
                    in1=prodn.rearrange("p k a (b t) -> p k a b t", t=2)[:, :, :, :, 1],
                    op=ALU.add)
                nc.vector.tensor_reduce(
                    rel_dyn_cm[:, nh * 32: nh * 32 + 32].rearrange(
                        "p (k b) -> p k b", k=2),
                    prodh, AX.X, ALU.add)

        psB_cm.__exit__(None, None, None)
        psA_cm.__exit__(None, None, None)
        distp_cm.__exit__(None, None, None)
        work_cm.__exit__(None, None, None)

        # --- tail: dyn = selfd + rel_dyn; affector; output head ---
        tailp_cm = tc.tile_pool(name="tailp", bufs=1)
        tailp = tailp_cm.__enter__()
        tail_ps_cm = tc.tile_pool(name="tail_ps", bufs=2, space="PSUM")
        tail_ps = tail_ps_cm.__enter__()
        dyn = tailp.tile([32, NO], F32R)
        nc.vector.tensor_tensor(out=dyn, in0=selfd, in1=rel_dyn_cm, op=ALU.add)

        aff1 = tailp.tile([32, NO], F32R)
        for blk in range(NO // BLK):
            sl = slice(blk * BLK, (blk + 1) * BLK)
            ps = tail_ps.tile([32, BLK], F32, tag="ps_small")
            nc.tensor.matmul(ps, t_waff0.bitcast(F32R), dyn[:, sl].bitcast(F32R), start=True, stop=True)
            nc.scalar.activation(aff1[:, sl], ps, AF.Tanh, bias=t_baff0)
        aff2 = tailp.tile([32, NO], F32R)
        for blk in range(NO // BLK):
            sl = slice(blk * BLK, (blk + 1) * BLK)
            ps = tail_ps.tile([32, BLK], F32, tag="ps_small")
            nc.tensor.matmul(ps, t_waff1.bitcast(F32R), aff1[:, sl].bitcast(F32R), start=True, stop=True)
            tmp = tailp.tile([32, BLK], F32, tag="afftmp", bufs=2, name="tmp")
            nc.scalar.activation(tmp, ps, AF.Tanh, bias=t_baff1)
            nc.vector.tensor_tensor(out=aff2[:, sl], in0=tmp, in1=aff1[:, sl].bitcast(F32),
                                    op=ALU.add)
        stack = tailp.tile([64, NO], F32R)
        for blk in range(NO // BLK):
            sl = slice(blk * BLK, (blk + 1) * BLK)
            ps = tail_ps.tile([32, BLK], F32, tag="ps_small")
            nc.tensor.matmul(ps, t_waff2.bitcast(F32R), aff2[:, sl].bitcast(F32R), start=True, stop=True)
            nc.scalar.activation(stack[0:32, sl], ps, AF.Identity, bias=t_baff2)
        nc.sync.dma_start(out=stack[32:64], in_=s2)

        res_cm = tailp.tile([32, NO], F32)
        for blk in range(NO // BLK):
            sl = slice(blk * BLK, (blk + 1) * BLK)
            ps = tail_ps.tile([32, BLK], F32, tag="ps_small")
            nc.tensor.matmul(ps, t_wout0.bitcast(F32R), stack[:, sl].bitcast(F32R), start=True, stop=True)
            o1 = tailp.tile([32, BLK], F32R, tag="o1", bufs=2, name="o1")
            nc.scalar.activation(o1, ps, AF.Tanh, bias=t_bout0)
            ps2 = tail_ps.tile([32, BLK], F32, tag="ps_small")
            nc.tensor.matmul(ps2, t_wout1.bitcast(F32R), o1.bitcast(F32R), start=True, stop=True)
            nc.vector.scalar_tensor_tensor(
                out=res_cm[:, sl], in0=ps2, scalar=t_bout1, in1=o1.bitcast(F32),
                op0=ALU.add, op1=ALU.add)

        for k in range(NO // 128):
            pst2 = tail_ps.tile([128, 32], F32, tag="ps_tr2", name="pst2")
            nc.tensor.transpose(pst2, res_cm[:, 128 * k:128 * (k + 1)],
                                t_iden[0:32, 0:32])
            o_sb = tailp.tile([128, 32], F32, tag="o_sb", bufs=3, name="o_sb")
            nc.scalar.activation(o_sb, pst2, AF.Copy)
            nc.sync.dma_start(
                out=out_d[4 * k:4 * k + 4].rearrange("a o c -> (a o) c"),
                in_=o_sb)
        tail_ps_cm.__exit__(None, None, None)
        tailp_cm.__exit__(None, None, None)

    nc.compile()
    return nc


_PROG = None


def _get_program():
    global _PROG
    if _PROG is None:
        _PROG = _build_program()
    return _PROG


def _prep_weights(inp):
    g = lambda k: np.asarray(inp[k], dtype=np.float32)
    w_s2_base = g("state_enc_w").copy()
    b_s2 = g("state_enc_b").copy()
    # s2 keeps raw channels 0-1: overwrite first two output cols with identity
    w_s2_base[:, 0:2] = 0.0
    w_s2_base[0, 0] = 1.0
    w_s2_base[1, 1] = 1.0
    b_s2[0:2] = 0.0
    w_s2 = np.zeros((128, 32), np.float32)
    for q in range(4):
        w_s2[32 * q:32 * q + 16] = w_s2_base

    rel_w0, att_w0 = g("rel_w0"), g("att_w0")   # (65, 64) each
    wi_cat = np.concatenate([rel_w0[:32], att_w0[:32]], axis=1)      # (32,128)
    wj_cat = np.concatenate([rel_w0[32:64], att_w0[32:64]], axis=1)  # (32,128)
    wd_cat = np.concatenate([rel_w0[64:65], att_w0[64:65]], axis=1)  # (1,128)
    b_l1 = np.concatenate([g("rel_b0"), g("att_b0")])                # (128,)

    w1_cat = np.zeros((128, 64), np.float32)
    w1_cat[0:64, 0:32] = g("rel_w1")
    w1_cat[64:128, 32:64] = g("att_w1")
    b_l2c = np.concatenate([g("rel_b1"), g("att_b1")])
    w2_cat = np.zeros((64, 33), np.float32)
    w2_cat[0:32, 0:32] = g("rel_w2") + np.eye(32, dtype=np.float32)
    w2_cat[32:64, 32:33] = g("att_w2")
    b3_relc = g("rel_b2")

    diagneg = np.zeros((33, 2 * BLK), np.float32)
    for ih in range(2):
        m = np.zeros((16, 32), np.float32)
        for il in range(16):
            m[il, ih * 16 + il] = 1.0
        diagneg[32, ih * BLK:(ih + 1) * BLK] = m.reshape(-1)
    ones2b = np.zeros((33, 32), np.float32)
    ones2b[32] = 1.0
    negrow = np.zeros((33, 32), np.float32)
    negrow[32] = -80.0

    col = lambda v: np.ascontiguousarray(v.reshape(-1, 1), dtype=np.float32)
    return {
        "w_s2": w_s2, "b_s2": col(b_s2),
        "wi_cat": wi_cat, "wj_cat": wj_cat, "wd_cat": wd_cat,
        "b_l1": col(b_l1), "w1_cat": w1_cat, "b_l2c": col(b_l2c),
        "w2_cat": w2_cat, "b3_relc": col(np.concatenate([g("rel_b2"), [0.0]]).astype(np.float32)),
        "b3_att": np.full((32, 1), float(g("att_b2").reshape(-1)[0]), np.float32),
        "diagneg": diagneg, "ones2b": ones2b,
        "negrow": negrow,
        "w_self0": g("self_w0"), "b_self0": col(g("self_b0")),
        "w_self1": g("self_w1"), "b_self1": col(g("self_b1")),
        "w_aff0": g("aff_w0"), "b_aff0": col(g("aff_b0")),
        "w_aff1": g("aff_w1"), "b_aff1": col(g("aff_b1")),
        "w_aff2": g("aff_w2"), "b_aff2": col(g("aff_b2")),
        "w_out0": g("out_w0"), "b_out0": col(g("out_b0")),
        "w_out1": g("out_w1"), "b_out1": col(g("out_b1")),
        "iden": np.eye(128, dtype=np.float32),
    }


def kernel(**inputs) -> np.ndarray:
    from concourse.bass_utils import run_bass_kernel_spmd

    nc = _get_program()
    weights = _prep_weights(inputs)
    s = np.asarray(inputs["s"], dtype=np.float32)
    in_maps = []
    for core in range(NCORES):
        m = dict(weights)
        m["s3d"] = np.ascontiguousarray(s[core * NP:(core + 1) * NP])
        in_maps.append(m)
    res = run_bass_kernel_spmd(nc, in_maps, list(range(NCORES)))
    out = np.concatenate([res.results[i]["out"] for i in range(NCORES)], axis=0)
    return out.astype(np.float32)



# revision 2
# speedup vs baseline: 1.7092x; 1.7092x over previous
"""Trainium2 Bass kernel for nn_Dynamics (GNN message passing), v2.

Data-parallel over batch n=1024 across 8 NeuronCores (NP=128 rows each).
Channel-major on-chip layout: (channels, batch*obj).

Key restructure vs v1: the rel tower's last Linear is lifted OUT of the
pair loop via linearity of the attention-weighted sum:
    rel_dyn = sum_j (W2^T rel2 + rel2 + b) * att
            = (W2 + I)^T g  +  b * A,   g = sum_j rel2*att, A = sum_j att
so the pair loop only needs: L1 (3 accum matmuls), L2 (K=128), the att
logit matmul (M=33, output replicated across partitions for free), one
exp evac, one multiply, one reduce. The all-ones row for A comes from the
L2 bias (zero weights + bias 1.0 -> relu -> 1). Diagonal (j==i) pairs are
removed post-hoc with stride-33 diagonal views on gpsimd.

Per 1024-pair row (steady state): PE 10 matmuls (2133ns), Act relu1+exp
(1992ns), DVE relu2 x2 + prod (1909ns), Pool reduce+diag (1833ns); the PE
stream is software-pipelined across rows so matmuls never wait on
same-row evacuations.
"""

import numpy as np
import ml_dtypes
from contextlib import ExitStack

import concourse.bass as bass
import concourse.mybir as mybir
import concourse.tile as tile
from concourse import bacc

F32 = mybir.dt.float32
F32R = mybir.dt.float32r
BF16 = mybir.dt.bfloat16
AF = mybir.ActivationFunctionType
ALU = mybir.AluOpType
AX = mybir.AxisListType

N = 1024
NOBJ = 32
CL = 32
NCORES = 8
NP = N // NCORES          # 128 batch rows per core
NO = NP * NOBJ            # 4096 objects per core
BLK = 512                 # pairs per PSUM bank (fp32)
NCHUNK = 32               # dist stream chunks of 4 batch rows
CHUNK_N = NP // NCHUNK    # 8

# M-layout of the fused L2 output (98 rows):
#   0:32   att2 (att tower hidden)
#   32:64  zero pad (so the ones/rel block lands at partition 64)
#   64     ones row (bias 1.0, zero weights) -> A accumulator
#   65:97  rel2 (rel tower hidden)
M2 = 98
OFS = 64                  # partition offset of [ones; rel2] block


def _build_program():
    nc = bacc.Bacc("TRN2", target_bir_lowering=False, debug=False)

    def din(name, shape, dt=F32):
        return nc.dram_tensor(name, list(shape), dt, kind="ExternalInput").ap()

    s3d = din("s3d", (NP, NOBJ, 16))
    # pack32: [wi(0:128) | wj(128:256) | self0 self1 aff0 aff1 aff2 out0a
    #          out0b out1 (256:576 in 32-col strips) | 8 biases (576:584) |
    #          wd on row 0 (584:712)]
    pack32 = din("pack32", (32, 712))
    # pack128: [w_s2(0:32) | b_l1(32) | w1_aug(33:131) | b2_aug(131) |
    #           b3_att(132) | w2_g rows 64:97 (133:165)]
    pack128 = din("pack128", (128, 165))
    w_att2 = din("w_att2", (32, 33), BF16)  # att_w2 replicated 33x
    iden = din("iden", (128, 128))

    out_d = nc.dram_tensor("out", [NP, NOBJ, CL], F32, kind="ExternalOutput").ap()

    with tile.TileContext(nc) as tc, ExitStack() as ctx:
        const = ctx.enter_context(tc.tile_pool(name="const", bufs=1))

        def load_const(ap_in, shape, dt=F32):
            nm = f"t_{ap_in.name}"
            t = const.tile(list(shape), dt, name=nm, tag=nm)
            nc.gpsimd.dma_start(out=t, in_=ap_in)
            return t

        def load_const_at(ap_in, shape, ofs, dt=F32):
            nm = f"t_{ap_in.name}"
            t = const.tile([ofs + shape[0]] + list(shape[1:]), dt,
                           name=nm, tag=nm)
            nc.gpsimd.dma_start(out=t[ofs:], in_=ap_in)
            return t[ofs:]

        def load_const_r(ap_in, shape, ofs=0):
            # f32 DMA load + rounded copy to an f32r tile at partition `ofs`
            # (matmul fmap/weight must share the SBUF base partition)
            nm = f"t_{ap_in.name}"
            tf = const.tile([ofs + shape[0]] + list(shape[1:]), F32,
                            name=nm + "_f", tag=nm + "_f")
            nc.gpsimd.dma_start(out=tf[ofs:], in_=ap_in)
            tr = const.tile([ofs + shape[0]] + list(shape[1:]), F32R,
                            name=nm, tag=nm)
            nc.vector.tensor_copy(tr[ofs:], tf[ofs:])
            return tr[ofs:]

        # critical-path loads first: identity + state, spread over the
        # three DMA-capable queues (SP, Act, Pool); packed consts after.
        t_iden = const.tile([128, 128], F32, name="t_iden", tag="t_iden")
        s_pad = const.tile([NP, NOBJ, 32], F32)
        nc.gpsimd.memset(s_pad[:, :, 16:32], 0.0)
        nc.sync.dma_start(out=t_iden, in_=iden)
        nc.sync.dma_start(out=s_pad[0:48, :, 0:16], in_=s3d[0:48])
        nc.scalar.dma_start(out=s_pad[48:96, :, 0:16], in_=s3d[48:96])
        nc.gpsimd.dma_start(out=s_pad[96:128, :, 0:16], in_=s3d[96:128])

        p32f = const.tile([32, 712], F32, name="p32f", tag="p32f")
        nc.scalar.dma_start(out=p32f, in_=pack32)
        p128f = const.tile([128, 165], F32, name="p128f", tag="p128f")
        nc.gpsimd.dma_start(out=p128f, in_=pack128)
        t_wat2 = const.tile([32, 33], BF16, name="t_wat2", tag="t_wat2")
        nc.scalar.dma_start(out=t_wat2, in_=w_att2)

        p32r = const.tile([32, 712], F32R, name="p32r", tag="p32r")
        nc.vector.tensor_copy(p32r[:, 0:576], p32f[:, 0:576])
        nc.vector.tensor_copy(p32r[0:1, 584:712], p32f[0:1, 584:712])
        p128r = const.tile([128, 165], F32R, name="p128r", tag="p128r")
        nc.vector.tensor_copy(p128r, p128f)

        t_wi = p32r[:, 0:128]
        t_wj = p32r[:, 128:256]
        t_wself0 = p32r[:, 256:288]
        t_wself1 = p32r[:, 288:320]
        t_waff0 = p32r[:, 320:352]
        t_waff1 = p32r[:, 352:384]
        t_waff2 = p32r[:, 384:416]
        t_wout0a = p32r[:, 416:448]
        t_wout0b = p32r[:, 448:480]
        t_wout1 = p32r[:, 480:512]
        t_bs2 = p32f[:, 576:577]
        t_bself0 = p32f[:, 577:578]
        t_bself1 = p32f[:, 578:579]
        t_baff0 = p32f[:, 579:580]
        t_baff1 = p32f[:, 580:581]
        t_baff2 = p32f[:, 581:582]
        t_bout0 = p32f[:, 582:583]
        t_bout1 = p32f[:, 583:584]
        t_wd = p32r[0:1, 584:712]
        t_ws2 = p128r[:, 0:32]
        t_bl1 = p128f[:, 32:33]
        t_w1a = p128r[:, 33:131]
        t_b2a = p128f[0:M2, 131:132]
        t_b3a = p128f[:, 132:133]
        t_w2g = p128r[64:97, 133:165]
        t_iden = const.tile([128, 128], F32, name="t_iden", tag="t_iden")
        nc.sync.dma_start(out=t_iden, in_=iden)

        # persistent activations
        s2 = const.tile([32, NO], F32R)
        dist_tr = const.tile([NP, NOBJ * NOBJ], F32R)    # (128, 1024)
        g_cm = const.tile([M2, NO], F32R)        # rows 64:97 = [A; g]

        # --- prologue: s2 build, dist ---

        wsb_cm = tc.tile_pool(name="wsb", bufs=1)
        wsb = wsb_cm.__enter__()
        wps_cm = tc.tile_pool(name="wps", bufs=2, space="PSUM")
        wps = wps_cm.__enter__()

        # s2 = concat(pos, enc[2:]) channel-major via PE transposes.
        # chunk k: transpose (128n, 4o x 32c); then one 1024-wide f32r
        # matmul per 16-channel strip (objects o = 4k + ol).
        s_flat = s_pad.rearrange("p a b -> p (a b)")     # (128, 1024)
        xt_all = wsb.tile([128, 8, 128], F32R, tag="xt", name="xt_all")
        for k in range(8):
            pst = wps.tile([128, 128], F32, tag="p1", name="pst")
            nc.tensor.transpose(pst, s_flat[:, 128 * k:128 * (k + 1)], t_iden)
            nc.vector.tensor_copy(xt_all[:, k], pst)
        s2_on = s2.rearrange("p (n o) -> p o n", o=NOBJ)
        for ol in range(4):
            ps = wps.tile([32, 8, 128], F32, tag="p1", name="ps_enc")
            for h in range(2):
                nc.tensor.matmul(ps[:, 4 * h:4 * h + 4],
                                 t_ws2[32 * ol:32 * ol + 16],
                                 xt_all[32 * ol:32 * ol + 16, 4 * h:4 * h + 4],
                                 start=True, stop=True,
                                 tile_position=(32 * ol, 0))
            nc.scalar.activation(s2_on[:, ol:NOBJ:4], ps, AF.Identity,
                                 bias=t_bs2)

        # pairwise squared distances (n on partitions), fp32
        posT = s_pad.rearrange("p a b -> p b a")[:, 0:2]   # (128, 2, 32)
        diff = wsb.tile([NP, 2, NOBJ, NOBJ], F32, tag="tb", bufs=4, name="diff")
        nc.vector.tensor_tensor(
            out=diff,
            in0=posT.unsqueeze(3).broadcast_to([NP, 2, NOBJ, NOBJ]),
            in1=posT.unsqueeze(2).broadcast_to([NP, 2, NOBJ, NOBJ]),
            op=ALU.subtract)
        sq = wsb.tile([NP, 2, NOBJ, NOBJ], F32, tag="tb", bufs=4, name="sq")
        nc.vector.tensor_tensor(out=sq, in0=diff, in1=diff, op=ALU.mult)
        nc.vector.tensor_tensor(
            out=dist_tr.rearrange("p (a b) -> p a b", b=NOBJ),
            in0=sq[:, 0], in1=sq[:, 1], op=ALU.add)

        # --- pair loop: one batch row (1024 pairs) per iteration, SW-pipelined ---

        P1 = {}      # rolling per-stage tiles keyed by row
        PAIR1 = {}
        P2 = {}
        R2 = {}
        P3 = {}
        ATTW = {}
        PRODT = {}
        PRODH = {}
        DPROD = {}
        DISTF = {}

        def prefetch(c):
            if c < NCHUNK and c not in DISTF:
                df = wsb.tile([1, CHUNK_N * 1024], F32R, tag="dist_f",
                                bufs=2, name="df")
                nc.sync.dma_start(
                    out=df, in_=dist_tr[CHUNK_N * c: CHUNK_N * (c + 1)])
                DISTF[c] = df

        def stage_dma(r):
            # prefetch the NEXT dist chunk while this one is consumed
            if r % CHUNK_N == 0:
                prefetch(r // CHUNK_N + 1)

        def stage_l1(r):
            # L1: p1 = Wi^T x_i + Wj^T x_j + wd^T dist  over (128, 1024)
            p1 = wps.tile([128, 1024], F32, tag="p1", name="p1").rearrange(
                "p (a b c) -> p a b c", b=16, c=NOBJ)
            df = DISTF[r // CHUNK_N]
            nl = r % CHUNK_N
            for ih in range(2):
                xi = s2[:, r * 32 + ih * 16: r * 32 + ih * 16 + 16]
                xi = xi.unsqueeze(2).broadcast_to([32, 16, NOBJ])
                xj = s2[:, r * 32: r * 32 + 32]
                xj = xj.unsqueeze(1).broadcast_to([32, 16, NOBJ])
                drow = df[0:1, nl * 1024 + ih * BLK: nl * 1024 + (ih + 1) * BLK]
                nc.tensor.matmul(p1[:, ih], t_wi, xi, start=True, stop=False)
                nc.tensor.matmul(p1[:, ih], t_wj, xj, start=False, stop=False)
                nc.tensor.matmul(
                    p1[:, ih].rearrange("p a b -> p (a b)"),
                    t_wd, drow, start=False, stop=True)
            P1[r] = p1

        def stage_relu1(r):
            # DVE evac: pair1 = relu(p1 + b)
            pair1 = wsb.tile([128, 1024], F32R, tag="pair1", bufs=2, name="pair1")
            nc.vector.tensor_scalar(
                out=pair1, in0=P1[r].rearrange("p a b c -> p (a b c)"),
                scalar1=t_bl1, scalar2=0.0, op0=ALU.add, op1=ALU.max)
            del P1[r]
            PAIR1[r] = pair1

        def stage_l2(r):
            p2 = wps.tile([M2, 2, BLK], F32, tag="p2", bufs=1, name="p2")
            for ih in range(2):
                nc.tensor.matmul(p2[:, ih], t_w1a,
                                 PAIR1[r][:, ih * BLK:(ih + 1) * BLK],
                                 start=True, stop=True)
            del PAIR1[r]
            P2[r] = p2

        def stage_relu2(r):
            # Act evac, single (98, 1024) op
            r2 = wsb.tile([M2, 1024], BF16, tag="r2", bufs=3, name="r2")
            nc.scalar.activation(
                r2, P2[r].rearrange("p a b -> p (a b)"), AF.Relu, bias=t_b2a)
            del P2[r]
            R2[r] = r2

        def stage_l3(r):
            # logit matmul: M=33 replicated output at partitions 64:97
            p3 = wps.tile([M2, 1024], F32, tag="p3", bufs=1, name="p3")
            for ih in range(2):
                nc.tensor.matmul(
                    p3[OFS:OFS + 33, ih * BLK:(ih + 1) * BLK], t_wat2,
                    R2[r][0:32, ih * BLK:(ih + 1) * BLK],
                    start=True, stop=True, tile_position=(0, OFS))
            P3[r] = p3

        def stage_exp(r):
            attw = wsb.tile([M2, 1024], BF16, tag="attw", bufs=3, name="attw")
            nc.scalar.activation(attw[OFS:OFS + 33], P3[r][OFS:OFS + 33],
                                 AF.Exp, bias=t_b3a[OFS:OFS + 33])
            del P3[r]
            ATTW[r] = attw

        def stage_prod(r):
            prodt = wsb.tile([M2, 1024], BF16, tag="prodt", bufs=2, name="prodt")
            nc.vector.tensor_tensor(
                out=prodt[OFS:OFS + 33], in0=R2[r][OFS:OFS + 33],
                in1=ATTW[r][OFS:OFS + 33], op=ALU.mult)
            PRODT[r] = prodt

        def stage_diagmult(r):
            dprod_t = wsb.tile([OFS + 33, 32], F32R, tag="dprod", bufs=3,
                                name="dprod")
            nc.gpsimd.tensor_tensor(
                out=dprod_t[OFS:OFS + 33], in0=R2[r][OFS:OFS + 33, 0:1024:33],
                in1=ATTW[r][OFS:OFS + 33, 0:1024:33], op=ALU.mult)
            DPROD[r] = dprod_t

        def stage_halve(r):
            # two binary-tree halving adds over j on gpsimd
            p4 = PRODT[r][OFS:OFS + 33].rearrange("p (a b) -> p a b", b=NOBJ)
            ph_t = wsb.tile([OFS + 33, 32, 16], F32, tag="prodh1", bufs=2,
                             name="ph1")
            nc.gpsimd.tensor_tensor(
                out=ph_t[OFS:OFS + 33], in0=p4[:, :, 0:16], in1=p4[:, :, 16:32],
                op=ALU.add)
            ph2_t = wsb.tile([OFS + 33, 32, 8], F32, tag="prodh2", bufs=2,
                              name="ph2")
            nc.gpsimd.tensor_tensor(
                out=ph2_t[OFS:OFS + 33], in0=ph_t[OFS:OFS + 33, :, 0:8],
                in1=ph_t[OFS:OFS + 33, :, 8:16], op=ALU.add)
            del PRODT[r]
            PRODH[r] = ph2_t

        def stage_reduce(r):
            # g[:, 32 cols of row r] = sum_j prod; then subtract diagonal
            gsl = g_cm[OFS:OFS + 33, r * 32:(r + 1) * 32]
            with nc.allow_low_precision(reason="f32r g accumulation (8 terms)"):
                nc.vector.tensor_reduce(
                    gsl, PRODH[r][OFS:OFS + 33], AX.X, ALU.add)
            del PRODH[r]

        def stage_diagsub(r):
            gsl = g_cm[OFS:OFS + 33, r * 32:(r + 1) * 32]
            nc.gpsimd.tensor_tensor(out=gsl, in0=gsl.bitcast(F32),
                                    in1=DPROD[r][OFS:OFS + 33].bitcast(F32),
                                    op=ALU.subtract)
            del DPROD[r]

        # software pipeline: stages of row r execute in iteration r+lag.
        # per-iteration PE order is l3(r-3), l1(r), l2(r-1) so each matmul's
        # producer evac finished at least one iteration earlier.
        STAGES = [
            (0, stage_dma),
            (3, stage_l3),
            (0, stage_l1),
            (1, stage_relu1),
            (1, stage_l2),
            (2, stage_relu2),
            (3, stage_exp),
            (4, stage_prod),
            (4, stage_diagmult),
            (5, stage_halve),
            (6, stage_reduce),
            (6, stage_diagsub),
        ]
        prefetch(0)
        DEPTH = 7
        for it in range(NP + DEPTH - 1):
            for lag, fn in STAGES:
                r = it - lag
                if 0 <= r < NP:
                    fn(r)

        # --- tail: rel_dyn from g; dyn; affector (aff3 folded into the
        # out-head weights); 2-block (32, 1024) psum evacs ---
        W = 1024
        h1 = wsb.tile([32, NO], F32R, tag="tb", bufs=4, name="h1")
        selfd = wsb.tile([32, NO], F32, tag="tb", bufs=4, name="selfd")
        dyn = wsb.tile([32, NO], F32R, tag="tb", bufs=4, name="dyn")
        aff1 = wsb.tile([32, NO], F32R, tag="tb", bufs=4, name="aff1")
        aff2 = wsb.tile([32, NO], F32R, tag="tb", bufs=4, name="aff2")
        o1 = wsb.tile([32, NO], F32R, tag="tb", bufs=4, name="o1")
        res_cm = wsb.tile([32, NO], F32, tag="tb", bufs=4, name="res")

        def wsl(w):
            return slice(w * W, (w + 1) * W)

        for w in range(NO // W):
            ps = wps.tile([32, W], F32, tag=["p1", "p2", "p3", "p1"][w], bufs=[2, 1, 1, 2][w], name="psa")
            for h in range(2):
                hs = slice(w * W + h * BLK, w * W + (h + 1) * BLK)
                nc.tensor.matmul(ps[:, h * BLK:(h + 1) * BLK], t_wself0,
                                 s2[:, hs], start=True, stop=True)
            nc.scalar.activation(h1[:, wsl(w)], ps, AF.Relu, bias=t_bself0)
        for w in range(NO // W):
            ps = wps.tile([32, W], F32, tag=["p1", "p2", "p3", "p1"][w], bufs=[2, 1, 1, 2][w], name="psb")
            for h in range(2):
                hs = slice(w * W + h * BLK, w * W + (h + 1) * BLK)
                nc.tensor.matmul(ps[:, h * BLK:(h + 1) * BLK], t_wself1,
                                 h1[:, hs], start=True, stop=True)
            nc.vector.scalar_tensor_tensor(
                out=selfd[:, wsl(w)], in0=ps, scalar=t_bself1,
                in1=h1[:, wsl(w)].bitcast(F32), op0=ALU.add, op1=ALU.add)
        for w in range(NO // W):
            ps = wps.tile([32, W], F32, tag=["p1", "p2", "p3", "p1"][w], bufs=[2, 1, 1, 2][w], name="psc")
            for h in range(2):
                hs = slice(w * W + h * BLK, w * W + (h + 1) * BLK)
                nc.tensor.matmul(ps[:, h * BLK:(h + 1) * BLK], t_w2g,
                                 g_cm[OFS:OFS + 33, hs],
                                 start=True, stop=True,
                                 tile_position=(OFS, 0))
            nc.vector.scalar_tensor_tensor(
                out=dyn[:, wsl(w)], in0=ps, scalar=0.0,
                in1=selfd[:, wsl(w)], op0=ALU.add, op1=ALU.add)
        for w in range(NO // W):
            ps = wps.tile([32, W], F32, tag=["p1", "p2", "p3", "p1"][w], bufs=[2, 1, 1, 2][w], name="psd")
            for h in range(2):
                hs = slice(w * W + h * BLK, w * W + (h + 1) * BLK)
                nc.tensor.matmul(ps[:, h * BLK:(h + 1) * BLK], t_waff0,
                                 dyn[:, hs], start=True, stop=True)
            nc.scalar.activation(aff1[:, wsl(w)], ps, AF.Tanh, bias=t_baff0)

        for w in range(NO // W):
            ps = wps.tile([32, W], F32, tag=["p1", "p2", "p3", "p1"][w], bufs=[2, 1, 1, 2][w], name="pse")
            for h in range(2):
                hs = slice(w * W + h * BLK, w * W + (h + 1) * BLK)
                nc.tensor.matmul(ps[:, h * BLK:(h + 1) * BLK], t_waff1,
                                 aff1[:, hs], start=True, stop=True)
            tmp = wsb.tile([32, W], F32R, tag="afftmp", bufs=2, name="tmp")
            nc.scalar.activation(tmp, ps, AF.Tanh, bias=t_baff1)
            nc.vector.tensor_tensor(out=aff2[:, wsl(w)],
                                    in0=tmp.bitcast(F32),
                                    in1=aff1[:, wsl(w)].bitcast(F32),
                                    op=ALU.add)
        for w in range(NO // W):
            ps = wps.tile([32, W], F32, tag=["p1", "p2", "p3", "p1"][w], bufs=[2, 1, 1, 2][w], name="psf")
            for h in range(2):
                hs = slice(w * W + h * BLK, w * W + (h + 1) * BLK)
                nc.tensor.matmul(ps[:, h * BLK:(h + 1) * BLK], t_wout0a,
                                 aff2[:, hs], start=True, stop=False)
                nc.tensor.matmul(ps[:, h * BLK:(h + 1) * BLK], t_wout0b,
                                 s2[:, hs], start=False, stop=True)
            nc.scalar.activation(o1[:, wsl(w)], ps, AF.Tanh, bias=t_bout0)
        for w in range(NO // W):
            ps = wps.tile([32, W], F32, tag=["p1", "p2", "p3", "p1"][w], bufs=[2, 1, 1, 2][w], name="psg")
            for h in range(2):
                hs = slice(w * W + h * BLK, w * W + (h + 1) * BLK)
                nc.tensor.matmul(ps[:, h * BLK:(h + 1) * BLK], t_wout1,
                                 o1[:, hs], start=True, stop=True)
            nc.vector.scalar_tensor_tensor(
                out=res_cm[:, wsl(w)], in0=ps, scalar=t_bout1,
                in1=o1[:, wsl(w)].bitcast(F32), op0=ALU.add, op1=ALU.add)

        # transpose back to (n, o, c) and store
        for b in range(NO // BLK):
            pst2 = wps.tile([128, 128], F32, tag="p1", name="pst2")
            for k in range(4):
                nc.tensor.transpose(
                    pst2[:, 32 * k:32 * (k + 1)],
                    res_cm[:, BLK * b + 128 * k: BLK * b + 128 * (k + 1)],
                    t_iden[0:32, 0:32])
            o_sb = wsb.tile([128, 128], F32, tag="o_sb", bufs=2, name="o_sb")
            nc.scalar.activation(o_sb, pst2, AF.Copy)
            nc.sync.dma_start(
                out=out_d[16 * b:16 * b + 16].rearrange(
                    "a o c -> (a o) c").rearrange("(k j) c -> j k c", k=4),
                in_=o_sb.rearrange("p (k c) -> p k c", c=32))
        wps_cm.__exit__(None, None, None)
        wsb_cm.__exit__(None, None, None)

    nc.compile()
    return nc


_PROG = None


def _get_program():
    global _PROG
    if _PROG is None:
        _PROG = _build_program()
    return _PROG


def _prep_weights(inp):
    g = lambda k: np.asarray(inp[k], dtype=np.float32)
    bf = lambda a: np.ascontiguousarray(a, dtype=ml_dtypes.bfloat16)
    col = lambda v: np.ascontiguousarray(
        np.asarray(v, np.float32).reshape(-1, 1), dtype=np.float32)

    w_s2_base = g("state_enc_w").copy()
    b_s2 = g("state_enc_b").copy()
    w_s2_base[:, 0:2] = 0.0
    w_s2_base[0, 0] = 1.0
    w_s2_base[1, 1] = 1.0
    b_s2[0:2] = 0.0
    w_s2 = np.zeros((128, 32), np.float32)
    for q in range(4):
        w_s2[32 * q:32 * q + 16] = w_s2_base

    rel_w0, att_w0 = g("rel_w0"), g("att_w0")   # (65, 64) each
    wi_cat = np.concatenate([rel_w0[:32], att_w0[:32]], axis=1)      # (32,128)
    wj_cat = np.concatenate([rel_w0[32:64], att_w0[32:64]], axis=1)  # (32,128)
    wd_cat = np.concatenate([rel_w0[64:65], att_w0[64:65]], axis=1)  # (1,128)
    b_l1 = np.concatenate([g("rel_b0"), g("att_b0")])                # (128,)

    # L2 fused: M-layout [att2(0:32) | pad(32:64) | ones(64) | rel2(65:97)]
    w1_aug = np.zeros((128, M2), np.float32)
    w1_aug[64:128, 0:32] = g("att_w1")
    w1_aug[0:64, 65:97] = g("rel_w1")
    b2_aug = np.zeros((M2,), np.float32)
    b2_aug[0:32] = g("att_b1")
    b2_aug[64] = 1.0
    b2_aug[65:97] = g("rel_b1")

    w_att2 = np.repeat(g("att_w2"), 33, axis=1)                      # (32,33)
    b3_att = np.zeros((M2,), np.float32)
    b3_att[OFS:OFS + 33] = float(g("att_b2").reshape(-1)[0])

    w2_g = np.zeros((33, 32), np.float32)
    w2_g[0] = g("rel_b2")
    w2_g[1:33] = g("rel_w2") + np.eye(32, dtype=np.float32)

    f = lambda a: np.ascontiguousarray(a, np.float32)
    pack32 = np.zeros((32, 712), np.float32)
    pack32[:, 0:128] = wi_cat
    pack32[:, 128:256] = wj_cat
    for i, w in enumerate(["self_w0", "self_w1", "aff_w0", "aff_w1",
                           "aff_w2"]):
        pack32[:, 256 + 32 * i:288 + 32 * i] = g(w)
    # aff3 is linear: fold aff_w2/aff_b2 into the out-head first layer
    pack32[:, 416:448] = g("aff_w2") @ g("out_w0")[0:32]
    pack32[:, 448:480] = g("out_w0")[32:64]
    pack32[:, 480:512] = g("out_w1")
    b_out0_f = g("out_b0") + g("aff_b2") @ g("out_w0")[0:32]
    for i, b in enumerate([b_s2, g("self_b0"), g("self_b1"), g("aff_b0"),
                           g("aff_b1"), g("aff_b2"), b_out0_f,
                           g("out_b1")]):
        pack32[:, 576 + i] = np.asarray(b, np.float32).reshape(-1)
    pack32[0, 584:712] = wd_cat[0]

    pack128 = np.zeros((128, 165), np.float32)
    pack128[:, 0:32] = w_s2
    pack128[:, 32] = b_l1
    pack128[:, 33:131] = w1_aug
    pack128[0:M2, 131] = b2_aug
    pack128[0:M2, 132] = b3_att
    pack128[64:97, 133:165] = w2_g

    return {
        "pack32": pack32, "pack128": pack128,
        "w_att2": bf(w_att2),
        "iden": np.eye(128, dtype=np.float32),
    }


def kernel(**inputs) -> np.ndarray:
    from concourse.bass_utils import run_bass_kernel_spmd

    nc = _get_program()
    weights = _prep_weights(inputs)
    s = np.asarray(inputs["s"], dtype=np.float32)
    in_maps = []
    for core in range(NCORES):
        m = dict(weights)
        m["s3d"] = np.ascontiguousarray(s[core * NP:(core + 1) * NP])
        in_maps.append(m)
    res = run_bass_kernel_spmd(nc, in_maps, list(range(NCORES)))
    out = np.concatenate([res.results[i]["out"] for i in range(NCORES)], axis=0)
    return out.astype(np.float32)


# revision 3
# speedup vs baseline: 1.7179x; 1.0051x over previous
"""Trainium2 Bass kernel for nn_Dynamics (GNN message passing).

Data-parallel over batch n=1024 across 8 NeuronCores (NP=128 rows each).
Channel-major on-chip layout: (channels, batch*obj); mixed precision
(f32r matmuls, bf16 on the rel2/attention product path).

Key structure: the rel tower's last Linear is lifted OUT of the per-pair
work via linearity of the attention-weighted sum:
    rel_dyn = sum_j (W2^T rel2 + rel2 + b) * att
            = (W2 + I)^T g  +  b * A,   g = sum_j rel2*att, A = sum_j att
so each 1024-pair batch row needs only 10 matmuls (L1 x6 accumulated
from x_i / x_j broadcasts and a streamed distance row, L2 x2 fused
[att2|ones|rel2], logit x2 with the scalar output replicated across 33
PSUM partitions via the M tile_position - free on the PE): PE 2133 ns.
Evacs are balanced across Act (relu2, exp), DVE (relu1, product, final
reduce) and gpsimd (halving adds, diagonal removal via stride-33 views),
each just under the PE budget; the PE stream is software-pipelined
across rows (l3(r-3), l1(r), l2(r-1)) so matmuls never wait on same-row
evacuations, and all phases share one SBUF/PSUM pool so no inter-phase
drain barriers are emitted. The affector's final Linear is folded into
the output head host-side. TimelineSim: ~343 us vs 589 us baseline.
"""

import numpy as np
import ml_dtypes
from contextlib import ExitStack

import concourse.bass as bass
import concourse.mybir as mybir
import concourse.tile as tile
from concourse import bacc

F32 = mybir.dt.float32
F32R = mybir.dt.float32r
BF16 = mybir.dt.bfloat16
AF = mybir.ActivationFunctionType
ALU = mybir.AluOpType
AX = mybir.AxisListType

N = 1024
NOBJ = 32
CL = 32
NCORES = 8
NP = N // NCORES          # 128 batch rows per core
NO = NP * NOBJ            # 4096 objects per core
BLK = 512                 # pairs per PSUM bank (fp32)
NCHUNK = 32               # dist stream chunks of 4 batch rows
CHUNK_N = NP // NCHUNK    # 8

# M-layout of the fused L2 output (98 rows):
#   0:32   att2 (att tower hidden)
#   32:64  zero pad (so the ones/rel block lands at partition 64)
#   64     ones row (bias 1.0, zero weights) -> A accumulator
#   65:97  rel2 (rel tower hidden)
M2 = 98
OFS = 64                  # partition offset of [ones; rel2] block


def _build_program():
    nc = bacc.Bacc("TRN2", target_bir_lowering=False, debug=False)

    def din(name, shape, dt=F32):
        return nc.dram_tensor(name, list(shape), dt, kind="ExternalInput").ap()

    s3d = din("s3d", (NP, NOBJ, 16))
    # pack32: [wi(0:128) | wj(128:256) | self0 self1 aff0 aff1 aff2 out0a
    #          out0b out1 (256:576 in 32-col strips) | 8 biases (576:584) |
    #          wd on row 0 (584:712)]
    pack32 = din("pack32", (32, 712))
    # pack128: [w_s2(0:32) | b_l1(32) | w1_aug(33:131) | b2_aug(131) |
    #           b3_att(132) | w2_g rows 64:97 (133:165)]
    pack128 = din("pack128", (128, 165))
    w_att2 = din("w_att2", (32, 33), BF16)  # att_w2 replicated 33x
    iden = din("iden", (128, 128))

    out_d = nc.dram_tensor("out", [NP, NOBJ, CL], F32, kind="ExternalOutput").ap()

    with tile.TileContext(nc) as tc, ExitStack() as ctx:
        const = ctx.enter_context(tc.tile_pool(name="const", bufs=1))

        def load_const(ap_in, shape, dt=F32):
            nm = f"t_{ap_in.name}"
            t = const.tile(list(shape), dt, name=nm, tag=nm)
            nc.gpsimd.dma_start(out=t, in_=ap_in)
            return t

        def load_const_at(ap_in, shape, ofs, dt=F32):
            nm = f"t_{ap_in.name}"
            t = const.tile([ofs + shape[0]] + list(shape[1:]), dt,
                           name=nm, tag=nm)
            nc.gpsimd.dma_start(out=t[ofs:], in_=ap_in)
            return t[ofs:]

        def load_const_r(ap_in, shape, ofs=0):
            # f32 DMA load + rounded copy to an f32r tile at partition `ofs`
            # (matmul fmap/weight must share the SBUF base partition)
            nm = f"t_{ap_in.name}"
            tf = const.tile([ofs + shape[0]] + list(shape[1:]), F32,
                            name=nm + "_f", tag=nm + "_f")
            nc.gpsimd.dma_start(out=tf[ofs:], in_=ap_in)
            tr = const.tile([ofs + shape[0]] + list(shape[1:]), F32R,
                            name=nm, tag=nm)
            nc.vector.tensor_copy(tr[ofs:], tf[ofs:])
            return tr[ofs:]

        # critical-path loads first: identity + state, spread over the
        # three DMA-capable queues (SP, Act, Pool); packed consts after.
        t_iden = const.tile([128, 128], F32, name="t_iden", tag="t_iden")
        s_pad = const.tile([NP, NOBJ, 32], F32)
        nc.gpsimd.memset(s_pad[:, :, 16:32], 0.0)
        nc.sync.dma_start(out=t_iden, in_=iden)
        nc.sync.dma_start(out=s_pad[0:48, :, 0:16], in_=s3d[0:48])
        nc.scalar.dma_start(out=s_pad[48:96, :, 0:16], in_=s3d[48:96])
        nc.gpsimd.dma_start(out=s_pad[96:128, :, 0:16], in_=s3d[96:128])

        p32f = const.tile([32, 712], F32, name="p32f", tag="p32f")
        nc.scalar.dma_start(out=p32f, in_=pack32)
        p128f = const.tile([128, 165], F32, name="p128f", tag="p128f")
        nc.gpsimd.dma_start(out=p128f, in_=pack128)
        t_wat2 = const.tile([32, 33], BF16, name="t_wat2", tag="t_wat2")
        nc.scalar.dma_start(out=t_wat2, in_=w_att2)

        p32r = const.tile([32, 712], F32R, name="p32r", tag="p32r")
        nc.vector.tensor_copy(p32r[:, 0:576], p32f[:, 0:576])
        nc.vector.tensor_copy(p32r[0:1, 584:712], p32f[0:1, 584:712])
        p128r = const.tile([128, 165], F32R, name="p128r", tag="p128r")
        nc.vector.tensor_copy(p128r, p128f)

        t_wi = p32r[:, 0:128]
        t_wj = p32r[:, 128:256]
        t_wself0 = p32r[:, 256:288]
        t_wself1 = p32r[:, 288:320]
        t_waff0 = p32r[:, 320:352]
        t_waff1 = p32r[:, 352:384]
        t_waff2 = p32r[:, 384:416]
        t_wout0a = p32r[:, 416:448]
        t_wout0b = p32r[:, 448:480]
        t_wout1 = p32r[:, 480:512]
        t_bs2 = p32f[:, 576:577]
        t_bself0 = p32f[:, 577:578]
        t_bself1 = p32f[:, 578:579]
        t_baff0 = p32f[:, 579:580]
        t_baff1 = p32f[:, 580:581]
        t_baff2 = p32f[:, 581:582]
        t_bout0 = p32f[:, 582:583]
        t_bout1 = p32f[:, 583:584]
        t_wd = p32r[0:1, 584:712]
        t_ws2 = p128r[:, 0:32]
        t_bl1 = p128f[:, 32:33]
        t_w1a = p128r[:, 33:131]
        t_b2a = p128f[0:M2, 131:132]
        t_b3a = p128f[:, 132:133]
        t_w2g = p128r[64:97, 133:165]
        t_iden = const.tile([128, 128], F32, name="t_iden", tag="t_iden")
        nc.sync.dma_start(out=t_iden, in_=iden)

        # persistent activations
        s2 = const.tile([32, NO], F32R)
        dist_tr = const.tile([NP, NOBJ * NOBJ], F32R)    # (128, 1024)
        g_cm = const.tile([M2, NO], F32R)        # rows 64:97 = [A; g]

        # --- prologue: s2 build, dist ---

        wsb_cm = tc.tile_pool(name="wsb", bufs=1)
        wsb = wsb_cm.__enter__()
        wps_cm = tc.tile_pool(name="wps", bufs=2, space="PSUM")
        wps = wps_cm.__enter__()

        # warm the PE p-state during DMA waits: ~40 junk matmuls on the
        # identity keep the ramp counter running so the loop's first rows
        # run at full clock (results unread).
        for wu in range(28):
            pwu = wps.tile([128, 128], F32, tag="p1", name="pwu")
            nc.tensor.matmul(pwu, t_iden, t_iden, start=True, stop=True)

        # s2 = concat(pos, enc[2:]) channel-major via PE transposes.
        # chunk k: transpose (128n, 4o x 32c); then one 1024-wide f32r
        # matmul per 16-channel strip (objects o = 4k + ol).
        s_flat = s_pad.rearrange("p a b -> p (a b)")     # (128, 1024)
        xt_all = wsb.tile([128, 8, 128], F32R, tag="xt", name="xt_all")
        for k in range(8):
            pst = wps.tile([128, 128], F32, tag="p1", name="pst")
            nc.tensor.transpose(pst, s_flat[:, 128 * k:128 * (k + 1)], t_iden)
            nc.vector.tensor_copy(xt_all[:, k], pst)
        s2_on = s2.rearrange("p (n o) -> p o n", o=NOBJ)
        for ol in range(4):
            ps = wps.tile([32, 8, 128], F32, tag="p1", name="ps_enc")
            for h in range(2):
                nc.tensor.matmul(ps[:, 4 * h:4 * h + 4],
                                 t_ws2[32 * ol:32 * ol + 16],
                                 xt_all[32 * ol:32 * ol + 16, 4 * h:4 * h + 4],
                                 start=True, stop=True,
                                 tile_position=(32 * ol, 0))
            nc.scalar.activation(s2_on[:, ol:NOBJ:4], ps, AF.Identity,
                                 bias=t_bs2)

        # pairwise squared distances (n on partitions), fp32
        posT = s_pad.rearrange("p a b -> p b a")[:, 0:2]   # (128, 2, 32)
        diff = wsb.tile([NP, 2, NOBJ, NOBJ], F32, tag="tb", bufs=4, name="diff")
        nc.vector.tensor_tensor(
            out=diff,
            in0=posT.unsqueeze(3).broadcast_to([NP, 2, NOBJ, NOBJ]),
            in1=posT.unsqueeze(2).broadcast_to([NP, 2, NOBJ, NOBJ]),
            op=ALU.subtract)
        sq = wsb.tile([NP, 2, NOBJ, NOBJ], F32, tag="tb", bufs=4, name="sq")
        nc.vector.tensor_tensor(out=sq, in0=diff, in1=diff, op=ALU.mult)
        nc.vector.tensor_tensor(
            out=dist_tr.rearrange("p (a b) -> p a b", b=NOBJ),
            in0=sq[:, 0], in1=sq[:, 1], op=ALU.add)

        # --- pair loop: one batch row (1024 pairs) per iteration, SW-pipelined ---

        P1 = {}      # rolling per-stage tiles keyed by row
        PAIR1 = {}
        P2 = {}
        R2 = {}
        P3 = {}
        ATTW = {}
        PRODT = {}
        PRODH = {}
        DPROD = {}
        DISTF = {}

        def prefetch(c):
            if c < NCHUNK and c not in DISTF:
                df = wsb.tile([1, CHUNK_N * 1024], F32R, tag="dist_f",
                                bufs=2, name="df")
                nc.sync.dma_start(
                    out=df, in_=dist_tr[CHUNK_N * c: CHUNK_N * (c + 1)])
                DISTF[c] = df

        def stage_dma(r):
            # prefetch the NEXT dist chunk while this one is consumed
            if r % CHUNK_N == 0:
                prefetch(r // CHUNK_N + 1)

        def stage_l1(r):
            # L1: p1 = Wi^T x_i + Wj^T x_j + wd^T dist  over (128, 1024)
            p1 = wps.tile([128, 1024], F32, tag="p1", name="p1").rearrange(
                "p (a b c) -> p a b c", b=16, c=NOBJ)
            df = DISTF[r // CHUNK_N]
            nl = r % CHUNK_N
            for ih in range(2):
                xi = s2[:, r * 32 + ih * 16: r * 32 + ih * 16 + 16]
                xi = xi.unsqueeze(2).broadcast_to([32, 16, NOBJ])
                xj = s2[:, r * 32: r * 32 + 32]
                xj = xj.unsqueeze(1).broadcast_to([32, 16, NOBJ])
                drow = df[0:1, nl * 1024 + ih * BLK: nl * 1024 + (ih + 1) * BLK]
                nc.tensor.matmul(p1[:, ih], t_wi, xi, start=True, stop=False)
                nc.tensor.matmul(p1[:, ih], t_wj, xj, start=False, stop=False)
                nc.tensor.matmul(
                    p1[:, ih].rearrange("p a b -> p (a b)"),
                    t_wd, drow, start=False, stop=True)
            P1[r] = p1

        def stage_relu1(r):
            # DVE evac: pair1 = relu(p1 + b)
            pair1 = wsb.tile([128, 1024], F32R, tag="pair1", bufs=2, name="pair1")
            nc.vector.tensor_scalar(
                out=pair1, in0=P1[r].rearrange("p a b c -> p (a b c)"),
                scalar1=t_bl1, scalar2=0.0, op0=ALU.add, op1=ALU.max)
            del P1[r]
            PAIR1[r] = pair1

        def stage_l2(r):
            p2 = wps.tile([M2, 2, BLK], F32, tag="p2", bufs=1, name="p2")
            for ih in range(2):
                nc.tensor.matmul(p2[:, ih], t_w1a,
                                 PAIR1[r][:, ih * BLK:(ih + 1) * BLK],
                                 start=True, stop=True)
            del PAIR1[r]
            P2[r] = p2

        def stage_relu2(r):
            # Act evac, single (98, 1024) op
            r2 = wsb.tile([M2, 1024], BF16, tag="r2", bufs=3, name="r2")
            nc.scalar.activation(
                r2, P2[r].rearrange("p a b -> p (a b)"), AF.Relu, bias=t_b2a)
            del P2[r]
            R2[r] = r2

        def stage_l3(r):
            # logit matmul: M=33 replicated output at partitions 64:97
            p3 = wps.tile([M2, 1024], F32, tag="p3", bufs=1, name="p3")
            for ih in range(2):
                nc.tensor.matmul(
                    p3[OFS:OFS + 33, ih * BLK:(ih + 1) * BLK], t_wat2,
                    R2[r][0:32, ih * BLK:(ih + 1) * BLK],
                    start=True, stop=True, tile_position=(0, OFS))
            P3[r] = p3

        def stage_exp(r):
            attw = wsb.tile([M2, 1024], BF16, tag="attw", bufs=3, name="attw")
            nc.scalar.activation(attw[OFS:OFS + 33], P3[r][OFS:OFS + 33],
                                 AF.Exp, bias=t_b3a[OFS:OFS + 33])
            del P3[r]
            ATTW[r] = attw

        def stage_prod(r):
            prodt = wsb.tile([M2, 1024], BF16, tag="prodt", bufs=2, name="prodt")
            nc.vector.tensor_tensor(
                out=prodt[OFS:OFS + 33], in0=R2[r][OFS:OFS + 33],
                in1=ATTW[r][OFS:OFS + 33], op=ALU.mult)
            PRODT[r] = prodt

        def stage_diagmult(r):
            dprod_t = wsb.tile([OFS + 33, 32], F32R, tag="dprod", bufs=3,
                                name="dprod")
            nc.gpsimd.tensor_tensor(
                out=dprod_t[OFS:OFS + 33], in0=R2[r][OFS:OFS + 33, 0:1024:33],
                in1=ATTW[r][OFS:OFS + 33, 0:1024:33], op=ALU.mult)
            DPROD[r] = dprod_t

        def stage_halve(r):
            # two binary-tree halving adds over j on gpsimd
            p4 = PRODT[r][OFS:OFS + 33].rearrange("p (a b) -> p a b", b=NOBJ)
            ph_t = wsb.tile([OFS + 33, 32, 16], F32, tag="prodh1", bufs=2,
                             name="ph1")
            nc.gpsimd.tensor_tensor(
                out=ph_t[OFS:OFS + 33], in0=p4[:, :, 0:16], in1=p4[:, :, 16:32],
                op=ALU.add)
            ph2_t = wsb.tile([OFS + 33, 32, 8], F32, tag="prodh2", bufs=2,
                              name="ph2")
            nc.gpsimd.tensor_tensor(
                out=ph2_t[OFS:OFS + 33], in0=ph_t[OFS:OFS + 33, :, 0:8],
                in1=ph_t[OFS:OFS + 33, :, 8:16], op=ALU.add)
            del PRODT[r]
            PRODH[r] = ph2_t

        def stage_reduce(r):
            # g[:, 32 cols of row r] = sum_j prod; then subtract diagonal
            gsl = g_cm[OFS:OFS + 33, r * 32:(r + 1) * 32]
            with nc.allow_low_precision(reason="f32r g accumulation (8 terms)"):
                nc.vector.tensor_reduce(
                    gsl, PRODH[r][OFS:OFS + 33], AX.X, ALU.add)
            del PRODH[r]

        def stage_diagsub(r):
            gsl = g_cm[OFS:OFS + 33, r * 32:(r + 1) * 32]
            nc.gpsimd.tensor_tensor(out=gsl, in0=gsl.bitcast(F32),
                                    in1=DPROD[r][OFS:OFS + 33].bitcast(F32),
                                    op=ALU.subtract)
            del DPROD[r]

        # software pipeline: stages of row r execute in iteration r+lag.
        # per-iteration PE order is l3(r-3), l1(r), l2(r-1) so each matmul's
        # producer evac finished at least one iteration earlier.
        STAGES = [
            (0, stage_dma),
            (3, stage_l3),
            (0, stage_l1),
            (1, stage_relu1),
            (1, stage_l2),
            (2, stage_relu2),
            (3, stage_exp),
            (4, stage_prod),
            (4, stage_diagmult),
            (5, stage_halve),
            (6, stage_reduce),
            (6, stage_diagsub),
        ]
        prefetch(0)
        DEPTH = 7
        for it in range(NP + DEPTH - 1):
            for lag, fn in STAGES:
                r = it - lag
                if 0 <= r < NP:
                    fn(r)

        # --- tail: rel_dyn from g; dyn; affector (aff3 folded into the
        # out-head weights); 2-block (32, 1024) psum evacs ---
        W = 1024
        h1 = wsb.tile([32, NO], F32R, tag="tb", bufs=4, name="h1")
        selfd = wsb.tile([32, NO], F32, tag="tb", bufs=4, name="selfd")
        dyn = wsb.tile([32, NO], F32R, tag="tb", bufs=4, name="dyn")
        aff1 = wsb.tile([32, NO], F32R, tag="tb", bufs=4, name="aff1")
        aff2 = wsb.tile([32, NO], F32R, tag="tb", bufs=4, name="aff2")
        o1 = wsb.tile([32, NO], F32R, tag="tb", bufs=4, name="o1")
        res_cm = wsb.tile([32, NO], F32, tag="tb", bufs=4, name="res")

        def wsl(w):
            return slice(w * W, (w + 1) * W)

        for w in range(NO // W):
            ps = wps.tile([32, W], F32, tag=["p1", "p2", "p3", "p1"][w], bufs=[2, 1, 1, 2][w], name="psa")
            for h in range(2):
                hs = slice(w * W + h * BLK, w * W + (h + 1) * BLK)
                nc.tensor.matmul(ps[:, h * BLK:(h + 1) * BLK], t_wself0,
                                 s2[:, hs], start=True, stop=True)
            nc.scalar.activation(h1[:, wsl(w)], ps, AF.Relu, bias=t_bself0)
        for w in range(NO // W):
            ps = wps.tile([32, W], F32, tag=["p1", "p2", "p3", "p1"][w], bufs=[2, 1, 1, 2][w], name="psb")
            for h in range(2):
                hs = slice(w * W + h * BLK, w * W + (h + 1) * BLK)
                nc.tensor.matmul(ps[:, h * BLK:(h + 1) * BLK], t_wself1,
                                 h1[:, hs], start=True, stop=True)
            nc.vector.scalar_tensor_tensor(
                out=selfd[:, wsl(w)], in0=ps, scalar=t_bself1,
                in1=h1[:, wsl(w)].bitcast(F32), op0=ALU.add, op1=ALU.add)
        for w in range(NO // W):
            ps = wps.tile([32, W], F32, tag=["p1", "p2", "p3", "p1"][w], bufs=[2, 1, 1, 2][w], name="psc")
            for h in range(2):
                hs = slice(w * W + h * BLK, w * W + (h + 1) * BLK)
                nc.tensor.matmul(ps[:, h * BLK:(h + 1) * BLK], t_w2g,
                                 g_cm[OFS:OFS + 33, hs],
                                 start=True, stop=True,
                                 tile_position=(OFS, 0))
            nc.vector.scalar_tensor_tensor(
                out=dyn[:, wsl(w)], in0=ps, scalar=0.0,
                in1=selfd[:, wsl(w)], op0=ALU.add, op1=ALU.add)
        for w in range(NO // W):
            ps = wps.tile([32, W], F32, tag=["p1", "p2", "p3", "p1"][w], bufs=[2, 1, 1, 2][w], name="psd")
            for h in range(2):
                hs = slice(w * W + h * BLK, w * W + (h + 1) * BLK)
                nc.tensor.matmul(ps[:, h * BLK:(h + 1) * BLK], t_waff0,
                                 dyn[:, hs], start=True, stop=True)
            nc.scalar.activation(aff1[:, wsl(w)], ps, AF.Tanh, bias=t_baff0)

        for w in range(NO // W):
            ps = wps.tile([32, W], F32, tag=["p1", "p2", "p3", "p1"][w], bufs=[2, 1, 1, 2][w], name="pse")
            for h in range(2):
                hs = slice(w * W + h * BLK, w * W + (h + 1) * BLK)
                nc.tensor.matmul(ps[:, h * BLK:(h + 1) * BLK], t_waff1,
                                 aff1[:, hs], start=True, stop=True)
            tmp = wsb.tile([32, W], F32R, tag="afftmp", bufs=2, name="tmp")
            nc.scalar.activation(tmp, ps, AF.Tanh, bias=t_baff1)
            nc.vector.tensor_tensor(out=aff2[:, wsl(w)],
                                    in0=tmp.bitcast(F32),
                                    in1=aff1[:, wsl(w)].bitcast(F32),
                                    op=ALU.add)
        for w in range(NO // W):
            ps = wps.tile([32, W], F32, tag=["p1", "p2", "p3", "p1"][w], bufs=[2, 1, 1, 2][w], name="psf")
            for h in range(2):
                hs = slice(w * W + h * BLK, w * W + (h + 1) * BLK)
                nc.tensor.matmul(ps[:, h * BLK:(h + 1) * BLK], t_wout0a,
                                 aff2[:, hs], start=True, stop=False)
                nc.tensor.matmul(ps[:, h * BLK:(h + 1) * BLK], t_wout0b,
                                 s2[:, hs], start=False, stop=True)
            nc.scalar.activation(o1[:, wsl(w)], ps, AF.Tanh, bias=t_bout0)
        for w in range(NO // W):
            ps = wps.tile([32, W], F32, tag=["p1", "p2", "p3", "p1"][w], bufs=[2, 1, 1, 2][w], name="psg")
            for h in range(2):
                hs = slice(w * W + h * BLK, w * W + (h + 1) * BLK)
                nc.tensor.matmul(ps[:, h * BLK:(h + 1) * BLK], t_wout1,
                                 o1[:, hs], start=True, stop=True)
            nc.vector.scalar_tensor_tensor(
                out=res_cm[:, wsl(w)], in0=ps, scalar=t_bout1,
                in1=o1[:, wsl(w)].bitcast(F32), op0=ALU.add, op1=ALU.add)

        # transpose back to (n, o, c) and store, 4 blocks per group
        for gq in range(2):
            pst2 = wps.tile([128, 512], F32, tag="p1", name="pst2")
            for j in range(4):
                b = 4 * gq + j
                q, c = b % 4, b // 4
                for k in range(4):
                    nc.tensor.transpose(
                        pst2[:, 128 * j + 32 * k:128 * j + 32 * (k + 1)],
                        res_cm[:, BLK * b + 128 * k: BLK * b + 128 * (k + 1)],
                        t_iden[0:32, 0:32])
            o_sb = wsb.tile([128, 512], F32, tag="o_sb", bufs=2, name="o_sb")
            nc.scalar.activation(o_sb, pst2, AF.Copy)
            nc.sync.dma_start(
                out=out_d.rearrange("a o c -> (a o) c")[
                    2048 * gq:2048 * (gq + 1)].rearrange(
                    "(j k p) c -> p j k c", j=4, k=4),
                in_=o_sb.rearrange("p (j k c) -> p j k c", j=4, c=32))
        wps_cm.__exit__(None, None, None)
        wsb_cm.__exit__(None, None, None)

    nc.compile()
    return nc


_PROG = None


def _get_program():
    global _PROG
    if _PROG is None:
        _PROG = _build_program()
    return _PROG


def _prep_weights(inp):
    g = lambda k: np.asarray(inp[k], dtype=np.float32)
    bf = lambda a: np.ascontiguousarray(a, dtype=ml_dtypes.bfloat16)
    col = lambda v: np.ascontiguousarray(
        np.asarray(v, np.float32).reshape(-1, 1), dtype=np.float32)

    w_s2_base = g("state_enc_w").copy()
    b_s2 = g("state_enc_b").copy()
    w_s2_base[:, 0:2] = 0.0
    w_s2_base[0, 0] = 1.0
    w_s2_base[1, 1] = 1.0
    b_s2[0:2] = 0.0
    w_s2 = np.zeros((128, 32), np.float32)
    for q in range(4):
        w_s2[32 * q:32 * q + 16] = w_s2_base

    rel_w0, att_w0 = g("rel_w0"), g("att_w0")   # (65, 64) each
    wi_cat = np.concatenate([rel_w0[:32], att_w0[:32]], axis=1)      # (32,128)
    wj_cat = np.concatenate([rel_w0[32:64], att_w0[32:64]], axis=1)  # (32,128)
    wd_cat = np.concatenate([rel_w0[64:65], att_w0[64:65]], axis=1)  # (1,128)
    b_l1 = np.concatenate([g("rel_b0"), g("att_b0")])                # (128,)

    # L2 fused: M-layout [att2(0:32) | pad(32:64) | ones(64) | rel2(65:97)]
    w1_aug = np.zeros((128, M2), np.float32)
    w1_aug[64:128, 0:32] = g("att_w1")
    w1_aug[0:64, 65:97] = g("rel_w1")
    b2_aug = np.zeros((M2,), np.float32)
    b2_aug[0:32] = g("att_b1")
    b2_aug[64] = 1.0
    b2_aug[65:97] = g("rel_b1")

    w_att2 = np.repeat(g("att_w2"), 33, axis=1)                      # (32,33)
    b3_att = np.zeros((M2,), np.float32)
    b3_att[OFS:OFS + 33] = float(g("att_b2").reshape(-1)[0])

    w2_g = np.zeros((33, 32), np.float32)
    w2_g[0] = g("rel_b2")
    w2_g[1:33] = g("rel_w2") + np.eye(32, dtype=np.float32)

    f = lambda a: np.ascontiguousarray(a, np.float32)
    pack32 = np.zeros((32, 712), np.float32)
    pack32[:, 0:128] = wi_cat
    pack32[:, 128:256] = wj_cat
    for i, w in enumerate(["self_w0", "self_w1", "aff_w0", "aff_w1",
                           "aff_w2"]):
        pack32[:, 256 + 32 * i:288 + 32 * i] = g(w)
    # aff3 is linear: fold aff_w2/aff_b2 into the out-head first layer
    pack32[:, 416:448] = g("aff_w2") @ g("out_w0")[0:32]
    pack32[:, 448:480] = g("out_w0")[32:64]
    pack32[:, 480:512] = g("out_w1")
    b_out0_f = g("out_b0") + g("aff_b2") @ g("out_w0")[0:32]
    for i, b in enumerate([b_s2, g("self_b0"), g("self_b1"), g("aff_b0"),
                           g("aff_b1"), g("aff_b2"), b_out0_f,
                           g("out_b1")]):
        pack32[:, 576 + i] = np.asarray(b, np.float32).reshape(-1)
    pack32[0, 584:712] = wd_cat[0]

    pack128 = np.zeros((128, 165), np.float32)
    pack128[:, 0:32] = w_s2
    pack128[:, 32] = b_l1
    pack128[:, 33:131] = w1_aug
    pack128[0:M2, 131] = b2_aug
    pack128[0:M2, 132] = b3_att
    pack128[64:97, 133:165] = w2_g

    return {
        "pack32": pack32, "pack128": pack128,
        "w_att2": bf(w_att2),
        "iden": np.eye(128, dtype=np.float32),
    }


def kernel(**inputs) -> np.ndarray:
    from concourse.bass_utils import run_bass_kernel_spmd

    nc = _get_program()
    weights = _prep_weights(inputs)
    s = np.asarray(inputs["s"], dtype=np.float32)
    in_maps = []
    for core in range(NCORES):
        m = dict(weights)
        m["s3d"] = np.ascontiguousarray(s[core * NP:(core + 1) * NP])
        in_maps.append(m)
    res = run_bass_kernel_spmd(nc, in_maps, list(range(NCORES)))
    out = np.concatenate([res.results[i]["out"] for i in range(NCORES)], axis=0)
    return out.astype(np.float32)


# revision 4
# speedup vs baseline: 1.7428x; 1.0145x over previous
"""Trainium2 Bass kernel for nn_Dynamics (GNN message passing).

Data-parallel over batch n=1024 across 8 NeuronCores (NP=128 rows each).
Channel-major on-chip layout: (channels, batch*obj); mixed precision
(f32r matmuls, bf16 on the rel2/attention product path).

Key structure: the rel tower's last Linear is lifted OUT of the per-pair
work via linearity of the attention-weighted sum:
    rel_dyn = sum_j (W2^T rel2 + rel2 + b) * att
            = (W2 + I)^T g  +  b * A,   g = sum_j rel2*att, A = sum_j att
so each 1024-pair batch row needs only 10 matmuls (L1 x6 accumulated
from x_i / x_j broadcasts and a streamed distance row, L2 x2 fused
[att2|ones|rel2], logit x2 with the scalar output replicated across 33
PSUM partitions via the M tile_position - free on the PE): PE 2133 ns.
Evacs are balanced across Act (relu2, exp), DVE (relu1, product, final
reduce) and gpsimd (halving adds, diagonal removal via stride-33 views),
each just under the PE budget; the PE stream is software-pipelined
across rows (l3(r-3), l1(r), l2(r-1)) so matmuls never wait on same-row
evacuations, and all phases share one SBUF/PSUM pool so no inter-phase
drain barriers are emitted. The affector's final Linear is folded into
the output head host-side. TimelineSim: ~343 us vs 589 us baseline.
"""

import numpy as np
import ml_dtypes
from contextlib import ExitStack

import concourse.bass as bass
import concourse.mybir as mybir
import concourse.tile as tile
from concourse import bacc

F32 = mybir.dt.float32
F32R = mybir.dt.float32r
BF16 = mybir.dt.bfloat16
AF = mybir.ActivationFunctionType
ALU = mybir.AluOpType
AX = mybir.AxisListType

N = 1024
NOBJ = 32
CL = 32
NCORES = 8
NP = N // NCORES          # 128 batch rows per core
NO = NP * NOBJ            # 4096 objects per core
BLK = 512                 # pairs per PSUM bank (fp32)
NCHUNK = 32               # dist stream chunks of 4 batch rows
CHUNK_N = NP // NCHUNK    # 8

# M-layout of the fused L2 output (98 rows):
#   0:32   att2 (att tower hidden)
#   32:64  zero pad (so the ones/rel block lands at partition 64)
#   64     ones row (bias 1.0, zero weights) -> A accumulator
#   65:97  rel2 (rel tower hidden)
M2 = 98
OFS = 64                  # partition offset of [ones; rel2] block


def _build_program():
    nc = bacc.Bacc("TRN2", target_bir_lowering=False, debug=False)

    def din(name, shape, dt=F32):
        return nc.dram_tensor(name, list(shape), dt, kind="ExternalInput").ap()

    s3d = din("s3d", (NP, NOBJ, 16))
    # pack32: [wi(0:128) | wj(128:256) | self0 self1 aff0 aff1 aff2 out0a
    #          out0b out1 (256:576 in 32-col strips) | 8 biases (576:584) |
    #          wd on row 0 (584:712)]
    pack32 = din("pack32", (32, 712))
    # pack128: [w_s2(0:32) | b_l1(32) | w1_aug(33:131) | b2_aug(131) |
    #           b3_att(132) | w2_g rows 64:97 (133:165)]
    pack128 = din("pack128", (128, 165))
    w_att2 = din("w_att2", (32, 33), BF16)  # att_w2 replicated 33x
    iden = din("iden", (128, 128))

    out_d = nc.dram_tensor("out", [NP, NOBJ, CL], F32, kind="ExternalOutput").ap()

    with tile.TileContext(nc) as tc, ExitStack() as ctx:
        const = ctx.enter_context(tc.tile_pool(name="const", bufs=1))

        def load_const(ap_in, shape, dt=F32):
            nm = f"t_{ap_in.name}"
            t = const.tile(list(shape), dt, name=nm, tag=nm)
            nc.gpsimd.dma_start(out=t, in_=ap_in)
            return t

        def load_const_at(ap_in, shape, ofs, dt=F32):
            nm = f"t_{ap_in.name}"
            t = const.tile([ofs + shape[0]] + list(shape[1:]), dt,
                           name=nm, tag=nm)
            nc.gpsimd.dma_start(out=t[ofs:], in_=ap_in)
            return t[ofs:]

        def load_const_r(ap_in, shape, ofs=0):
            # f32 DMA load + rounded copy to an f32r tile at partition `ofs`
            # (matmul fmap/weight must share the SBUF base partition)
            nm = f"t_{ap_in.name}"
            tf = const.tile([ofs + shape[0]] + list(shape[1:]), F32,
                            name=nm + "_f", tag=nm + "_f")
            nc.gpsimd.dma_start(out=tf[ofs:], in_=ap_in)
            tr = const.tile([ofs + shape[0]] + list(shape[1:]), F32R,
                            name=nm, tag=nm)
            nc.vector.tensor_copy(tr[ofs:], tf[ofs:])
            return tr[ofs:]

        # critical-path loads first: identity + state, spread over the
        # three DMA-capable queues (SP, Act, Pool); packed consts after.
        t_iden = const.tile([128, 128], F32, name="t_iden", tag="t_iden")
        s_pad = const.tile([NP, NOBJ, 32], F32)
        nc.gpsimd.memset(s_pad[:, :, 16:32], 0.0)
        nc.sync.dma_start(out=t_iden, in_=iden)
        nc.sync.dma_start(out=s_pad[0:48, :, 0:16], in_=s3d[0:48])
        nc.scalar.dma_start(out=s_pad[48:96, :, 0:16], in_=s3d[48:96])
        nc.gpsimd.dma_start(out=s_pad[96:128, :, 0:16], in_=s3d[96:128])

        p32f = const.tile([32, 712], F32, name="p32f", tag="p32f")
        nc.scalar.dma_start(out=p32f, in_=pack32)
        p128f = const.tile([128, 165], F32, name="p128f", tag="p128f")
        nc.gpsimd.dma_start(out=p128f, in_=pack128)
        t_wat2 = const.tile([32, 33], BF16, name="t_wat2", tag="t_wat2")
        nc.scalar.dma_start(out=t_wat2, in_=w_att2)

        p32r = const.tile([32, 712], F32R, name="p32r", tag="p32r")
        nc.vector.tensor_copy(p32r[:, 0:576], p32f[:, 0:576])
        nc.vector.tensor_copy(p32r[0:1, 584:712], p32f[0:1, 584:712])
        p128r = const.tile([128, 165], F32R, name="p128r", tag="p128r")
        nc.vector.tensor_copy(p128r, p128f)

        t_wi = p32r[:, 0:128]
        t_wj = p32r[:, 128:256]
        t_wself0 = p32r[:, 256:288]
        t_wself1 = p32r[:, 288:320]
        t_waff0 = p32r[:, 320:352]
        t_waff1 = p32r[:, 352:384]
        t_waff2 = p32r[:, 384:416]
        t_wout0a = p32r[:, 416:448]
        t_wout0b = p32r[:, 448:480]
        t_wout1 = p32r[:, 480:512]
        t_bs2 = p32f[:, 576:577]
        t_bself0 = p32f[:, 577:578]
        t_bself1 = p32f[:, 578:579]
        t_baff0 = p32f[:, 579:580]
        t_baff1 = p32f[:, 580:581]
        t_baff2 = p32f[:, 581:582]
        t_bout0 = p32f[:, 582:583]
        t_bout1 = p32f[:, 583:584]
        t_wd = p32r[0:1, 584:712]
        t_ws2 = p128r[:, 0:32]
        t_bl1 = p128f[:, 32:33]
        t_w1a = p128r[:, 33:131]
        t_b2a = p128f[0:M2, 131:132]
        t_b3a = p128f[:, 132:133]
        t_w2g = p128r[64:97, 133:165]
        t_iden = const.tile([128, 128], F32, name="t_iden", tag="t_iden")
        nc.sync.dma_start(out=t_iden, in_=iden)

        # persistent activations
        s2 = const.tile([32, NO], F32R)
        dist_tr = const.tile([NP, NOBJ * NOBJ], F32R)    # (128, 1024)
        g_cm = const.tile([M2, NO], F32R)        # rows 64:97 = [A; g]

        # --- prologue: s2 build, dist ---

        wsb_cm = tc.tile_pool(name="wsb", bufs=1)
        wsb = wsb_cm.__enter__()
        wps_cm = tc.tile_pool(name="wps", bufs=2, space="PSUM")
        wps = wps_cm.__enter__()

        # warm the PE p-state during DMA waits: ~40 junk matmuls on the
        # identity keep the ramp counter running so the loop's first rows
        # run at full clock (results unread).
        for wu in range(28):
            pwu = wps.tile([128, 128], F32, tag="p1", name="pwu")
            nc.tensor.matmul(pwu, t_iden, t_iden, start=True, stop=True)

        # s2 = concat(pos, enc[2:]) channel-major via PE transposes.
        # chunk k: transpose (128n, 4o x 32c); then one 1024-wide f32r
        # matmul per 16-channel strip (objects o = 4k + ol).
        s_flat = s_pad.rearrange("p a b -> p (a b)")     # (128, 1024)
        xt_all = wsb.tile([128, 8, 128], F32R, tag="xt", name="xt_all")
        for k in range(8):
            pst = wps.tile([128, 128], F32, tag="p1", name="pst")
            nc.tensor.transpose(pst, s_flat[:, 128 * k:128 * (k + 1)], t_iden)
            nc.vector.tensor_copy(xt_all[:, k], pst)
        s2_on = s2.rearrange("p (n o) -> p o n", o=NOBJ)
        for ol in range(4):
            ps = wps.tile([32, 8, 128], F32, tag="p1", name="ps_enc")
            for h in range(2):
                nc.tensor.matmul(ps[:, 4 * h:4 * h + 4],
                                 t_ws2[32 * ol:32 * ol + 16],
                                 xt_all[32 * ol:32 * ol + 16, 4 * h:4 * h + 4],
                                 start=True, stop=True,
                                 tile_position=(32 * ol, 0))
            nc.scalar.activation(s2_on[:, ol:NOBJ:4], ps, AF.Identity,
                                 bias=t_bs2)

        # pairwise squared distances (n on partitions), fp32
        posT = s_pad.rearrange("p a b -> p b a")[:, 0:2]   # (128, 2, 32)
        diff = wsb.tile([NP, 2, NOBJ, NOBJ], F32, tag="tb", bufs=4, name="diff")
        nc.vector.tensor_tensor(
            out=diff,
            in0=posT.unsqueeze(3).broadcast_to([NP, 2, NOBJ, NOBJ]),
            in1=posT.unsqueeze(2).broadcast_to([NP, 2, NOBJ, NOBJ]),
            op=ALU.subtract)
        sq = wsb.tile([NP, 2, NOBJ, NOBJ], F32, tag="tb", bufs=4, name="sq")
        nc.vector.tensor_tensor(out=sq, in0=diff, in1=diff, op=ALU.mult)
        nc.vector.tensor_tensor(
            out=dist_tr.rearrange("p (a b) -> p a b", b=NOBJ),
            in0=sq[:, 0], in1=sq[:, 1], op=ALU.add)

        # --- pair loop: one batch row (1024 pairs) per iteration, SW-pipelined ---

        P1 = {}      # rolling per-stage tiles keyed by row
        PAIR1 = {}
        P2 = {}
        R2 = {}
        P3 = {}
        ATTW = {}
        PRODT = {}
        PRODH = {}
        DPROD = {}
        DISTF = {}

        def prefetch(c):
            if c < NCHUNK and c not in DISTF:
                df = wsb.tile([1, CHUNK_N * 1024], F32R, tag="dist_f",
                                bufs=2, name="df")
                nc.sync.dma_start(
                    out=df, in_=dist_tr[CHUNK_N * c: CHUNK_N * (c + 1)])
                DISTF[c] = df

        def stage_dma(r):
            # prefetch the NEXT dist chunk while this one is consumed
            if r % CHUNK_N == 0:
                prefetch(r // CHUNK_N + 1)

        def stage_l1(r):
            # L1: p1 = Wi^T x_i + Wj^T x_j + wd^T dist  over (128, 1024)
            p1 = wps.tile([128, 1024], F32, tag="p1", name="p1").rearrange(
                "p (a b c) -> p a b c", b=16, c=NOBJ)
            df = DISTF[r // CHUNK_N]
            nl = r % CHUNK_N
            for ih in range(2):
                xi = s2[:, r * 32 + ih * 16: r * 32 + ih * 16 + 16]
                xi = xi.unsqueeze(2).broadcast_to([32, 16, NOBJ])
                xj = s2[:, r * 32: r * 32 + 32]
                xj = xj.unsqueeze(1).broadcast_to([32, 16, NOBJ])
                drow = df[0:1, nl * 1024 + ih * BLK: nl * 1024 + (ih + 1) * BLK]
                nc.tensor.matmul(p1[:, ih], t_wi, xi, start=True, stop=False)
                nc.tensor.matmul(p1[:, ih], t_wj, xj, start=False, stop=False)
                nc.tensor.matmul(
                    p1[:, ih].rearrange("p a b -> p (a b)"),
                    t_wd, drow, start=False, stop=True)
            P1[r] = p1

        def stage_relu1(r):
            # DVE evac: pair1 = relu(p1 + b)
            pair1 = wsb.tile([128, 1024], F32R, tag="pair1", bufs=2, name="pair1")
            nc.vector.tensor_scalar(
                out=pair1, in0=P1[r].rearrange("p a b c -> p (a b c)"),
                scalar1=t_bl1, scalar2=0.0, op0=ALU.add, op1=ALU.max)
            del P1[r]
            PAIR1[r] = pair1

        def stage_l2(r):
            p2 = wps.tile([M2, 2, BLK], F32, tag="p2", bufs=1, name="p2")
            for ih in range(2):
                nc.tensor.matmul(p2[:, ih], t_w1a,
                                 PAIR1[r][:, ih * BLK:(ih + 1) * BLK],
                                 start=True, stop=True)
            del PAIR1[r]
            P2[r] = p2

        def stage_relu2(r):
            # Act evac, single (98, 1024) op
            r2 = wsb.tile([M2, 1024], BF16, tag="r2", bufs=3, name="r2")
            nc.scalar.activation(
                r2, P2[r].rearrange("p a b -> p (a b)"), AF.Relu, bias=t_b2a)
            del P2[r]
            R2[r] = r2

        def stage_l3(r):
            # logit matmul: M=33 replicated output at partitions 64:97
            p3 = wps.tile([M2, 1024], F32, tag="p3", bufs=1, name="p3")
            for ih in range(2):
                nc.tensor.matmul(
                    p3[OFS:OFS + 33, ih * BLK:(ih + 1) * BLK], t_wat2,
                    R2[r][0:32, ih * BLK:(ih + 1) * BLK],
                    start=True, stop=True, tile_position=(0, OFS))
            P3[r] = p3

        def stage_exp(r):
            attw = wsb.tile([M2, 1024], BF16, tag="attw", bufs=3, name="attw")
            nc.scalar.activation(attw[OFS:OFS + 33], P3[r][OFS:OFS + 33],
                                 AF.Exp, bias=t_b3a[OFS:OFS + 33])
            del P3[r]
            ATTW[r] = attw

        def stage_prod(r):
            prodt = wsb.tile([M2, 1024], BF16, tag="prodt", bufs=2, name="prodt")
            nc.vector.tensor_tensor(
                out=prodt[OFS:OFS + 33], in0=R2[r][OFS:OFS + 33],
                in1=ATTW[r][OFS:OFS + 33], op=ALU.mult)
            PRODT[r] = prodt

        def stage_diagmult(r):
            dprod_t = wsb.tile([OFS + 33, 32], F32R, tag="dprod", bufs=3,
                                name="dprod")
            nc.gpsimd.tensor_tensor(
                out=dprod_t[OFS:OFS + 33], in0=R2[r][OFS:OFS + 33, 0:1024:33],
                in1=ATTW[r][OFS:OFS + 33, 0:1024:33], op=ALU.mult)
            DPROD[r] = dprod_t

        def stage_halve(r):
            # two binary-tree halving adds over j on gpsimd
            p4 = PRODT[r][OFS:OFS + 33].rearrange("p (a b) -> p a b", b=NOBJ)
            ph_t = wsb.tile([OFS + 33, 32, 16], F32, tag="prodh1", bufs=2,
                             name="ph1")
            nc.gpsimd.tensor_tensor(
                out=ph_t[OFS:OFS + 33], in0=p4[:, :, 0:16], in1=p4[:, :, 16:32],
                op=ALU.add)
            ph2_t = wsb.tile([OFS + 33, 32, 8], F32, tag="prodh2", bufs=2,
                              name="ph2")
            nc.gpsimd.tensor_tensor(
                out=ph2_t[OFS:OFS + 33], in0=ph_t[OFS:OFS + 33, :, 0:8],
                in1=ph_t[OFS:OFS + 33, :, 8:16], op=ALU.add)
            del PRODT[r]
            PRODH[r] = ph2_t

        def stage_reduce(r):
            # g[:, 32 cols of row r] = sum_j prod; then subtract diagonal
            gsl = g_cm[OFS:OFS + 33, r * 32:(r + 1) * 32]
            with nc.allow_low_precision(reason="f32r g accumulation (8 terms)"):
                nc.vector.tensor_reduce(
                    gsl, PRODH[r][OFS:OFS + 33], AX.X, ALU.add)
            del PRODH[r]

        def stage_diagsub(r):
            gsl = g_cm[OFS:OFS + 33, r * 32:(r + 1) * 32]
            nc.gpsimd.tensor_tensor(out=gsl, in0=gsl.bitcast(F32),
                                    in1=DPROD[r][OFS:OFS + 33].bitcast(F32),
                                    op=ALU.subtract)
            del DPROD[r]

        # software pipeline: stages of row r execute in iteration r+lag.
        # per-iteration PE order is l3(r-3), l1(r), l2(r-1) so each matmul's
        # producer evac finished at least one iteration earlier.
        STAGES = [
            (0, stage_dma),
            (3, stage_l3),
            (0, stage_l1),
            (1, stage_relu1),
            (1, stage_l2),
            (2, stage_relu2),
            (3, stage_exp),
            (4, stage_prod),
            (4, stage_diagmult),
            (5, stage_halve),
            (6, stage_reduce),
            (6, stage_diagsub),
        ]
        prefetch(0)
        DEPTH = 7
        for it in range(NP + DEPTH - 1):
            for lag, fn in STAGES:
                r = it - lag
                if 0 <= r < NP:
                    fn(r)

        # --- tail: rel_dyn from g; dyn; affector (aff3 folded into the
        # out-head weights); 2-block (32, 1024) psum evacs ---
        W = 1024
        h1 = wsb.tile([32, NO], F32R, tag="tb", bufs=4, name="h1")
        selfd = wsb.tile([32, NO], F32, tag="tb", bufs=4, name="selfd")
        dyn = wsb.tile([32, NO], F32R, tag="tb", bufs=4, name="dyn")
        aff1 = wsb.tile([32, NO], F32R, tag="tb", bufs=4, name="aff1")
        aff2 = wsb.tile([32, NO], F32R, tag="tb", bufs=4, name="aff2")
        o1 = wsb.tile([32, NO], F32R, tag="tb", bufs=4, name="o1")
        res_cm = wsb.tile([32, NO], F32, tag="tb", bufs=4, name="res")

        def wsl(w):
            return slice(w * W, (w + 1) * W)

        for w in range(NO // W):
            ps = wps.tile([32, W], F32, tag=["p1", "p2", "p3", "p1"][w], bufs=[2, 1, 1, 2][w], name="psa")
            for h in range(2):
                hs = slice(w * W + h * BLK, w * W + (h + 1) * BLK)
                nc.tensor.matmul(ps[:, h * BLK:(h + 1) * BLK], t_wself0,
                                 s2[:, hs], start=True, stop=True)
            nc.vector.tensor_scalar(
                out=h1[:, wsl(w)], in0=ps, scalar1=t_bself0, scalar2=0.0,
                op0=ALU.add, op1=ALU.max)
        for w in range(NO // W):
            ps = wps.tile([32, W], F32, tag=["p1", "p2", "p3", "p1"][w], bufs=[2, 1, 1, 2][w], name="psb")
            for h in range(2):
                hs = slice(w * W + h * BLK, w * W + (h + 1) * BLK)
                nc.tensor.matmul(ps[:, h * BLK:(h + 1) * BLK], t_wself1,
                                 h1[:, hs], start=True, stop=True)
            nc.scalar.activation(selfd[:, wsl(w)], ps, AF.Identity,
                                 bias=t_bself1)
        for w in range(NO // W):
            ps = wps.tile([32, W], F32, tag=["p1", "p2", "p3", "p1"][w], bufs=[2, 1, 1, 2][w], name="psc")
            for h in range(2):
                hs = slice(w * W + h * BLK, w * W + (h + 1) * BLK)
                nc.tensor.matmul(ps[:, h * BLK:(h + 1) * BLK], t_w2g,
                                 g_cm[OFS:OFS + 33, hs],
                                 start=True, stop=True,
                                 tile_position=(OFS, 0))
            nc.vector.scalar_tensor_tensor(
                out=dyn[:, wsl(w)], in0=ps, scalar=0.0,
                in1=selfd[:, wsl(w)], op0=ALU.add, op1=ALU.add)
        for w in range(NO // W):
            ps = wps.tile([32, W], F32, tag=["p1", "p2", "p3", "p1"][w], bufs=[2, 1, 1, 2][w], name="psd")
            for h in range(2):
                hs = slice(w * W + h * BLK, w * W + (h + 1) * BLK)
                nc.tensor.matmul(ps[:, h * BLK:(h + 1) * BLK], t_waff0,
                                 dyn[:, hs], start=True, stop=True)
            nc.scalar.activation(aff1[:, wsl(w)], ps, AF.Tanh, bias=t_baff0)

        for w in range(NO // W):
            ps = wps.tile([32, W], F32, tag=["p1", "p2", "p3", "p1"][w], bufs=[2, 1, 1, 2][w], name="pse")
            for h in range(2):
                hs = slice(w * W + h * BLK, w * W + (h + 1) * BLK)
                nc.tensor.matmul(ps[:, h * BLK:(h + 1) * BLK], t_waff1,
                                 aff1[:, hs], start=True, stop=True)
            tmp = wsb.tile([32, W], F32R, tag="afftmp", bufs=2, name="tmp")
            nc.scalar.activation(tmp, ps, AF.Tanh, bias=t_baff1)
            nc.vector.tensor_tensor(out=aff2[:, wsl(w)],
                                    in0=tmp.bitcast(F32),
                                    in1=aff1[:, wsl(w)].bitcast(F32),
                                    op=ALU.add)
        for w in range(NO // W):
            ps = wps.tile([32, W], F32, tag=["p1", "p2", "p3", "p1"][w], bufs=[2, 1, 1, 2][w], name="psf")
            for h in range(2):
                hs = slice(w * W + h * BLK, w * W + (h + 1) * BLK)
                nc.tensor.matmul(ps[:, h * BLK:(h + 1) * BLK], t_wout0a,
                                 aff2[:, hs], start=True, stop=False)
                nc.tensor.matmul(ps[:, h * BLK:(h + 1) * BLK], t_wout0b,
                                 s2[:, hs], start=False, stop=True)
            nc.scalar.activation(o1[:, wsl(w)], ps, AF.Tanh, bias=t_bout0)
        for w in range(NO // W):
            ps = wps.tile([32, W], F32, tag=["p1", "p2", "p3", "p1"][w], bufs=[2, 1, 1, 2][w], name="psg")
            for h in range(2):
                hs = slice(w * W + h * BLK, w * W + (h + 1) * BLK)
                nc.tensor.matmul(ps[:, h * BLK:(h + 1) * BLK], t_wout1,
                                 o1[:, hs], start=True, stop=True)
            nc.vector.scalar_tensor_tensor(
                out=res_cm[:, wsl(w)], in0=ps, scalar=t_bout1,
                in1=o1[:, wsl(w)].bitcast(F32), op0=ALU.add, op1=ALU.add)

        # transpose back to (n, o, c) and store, 4 blocks per group
        for gq in range(2):
            pst2 = wps.tile([128, 512], F32, tag="p1", name="pst2")
            for j in range(4):
                b = 4 * gq + j
                q, c = b % 4, b // 4
                for k in range(4):
                    nc.tensor.transpose(
                        pst2[:, 128 * j + 32 * k:128 * j + 32 * (k + 1)],
                        res_cm[:, BLK * b + 128 * k: BLK * b + 128 * (k + 1)],
                        t_iden[0:32, 0:32])
            o_sb = wsb.tile([128, 512], F32, tag="o_sb", bufs=2, name="o_sb")
            nc.scalar.activation(o_sb, pst2, AF.Copy)
            nc.sync.dma_start(
                out=out_d.rearrange("a o c -> (a o) c")[
                    2048 * gq:2048 * (gq + 1)].rearrange(
                    "(j k p) c -> p j k c", j=4, k=4),
                in_=o_sb.rearrange("p (j k c) -> p j k c", j=4, c=32))
        wps_cm.__exit__(None, None, None)
        wsb_cm.__exit__(None, None, None)

    nc.compile()
    return nc


_PROG = None


def _get_program():
    global _PROG
    if _PROG is None:
        _PROG = _build_program()
    return _PROG


def _prep_weights(inp):
    g = lambda k: np.asarray(inp[k], dtype=np.float32)
    bf = lambda a: np.ascontiguousarray(a, dtype=ml_dtypes.bfloat16)
    col = lambda v: np.ascontiguousarray(
        np.asarray(v, np.float32).reshape(-1, 1), dtype=np.float32)

    w_s2_base = g("state_enc_w").copy()
    b_s2 = g("state_enc_b").copy()
    w_s2_base[:, 0:2] = 0.0
    w_s2_base[0, 0] = 1.0
    w_s2_base[1, 1] = 1.0
    b_s2[0:2] = 0.0
    w_s2 = np.zeros((128, 32), np.float32)
    for q in range(4):
        w_s2[32 * q:32 * q + 16] = w_s2_base

    rel_w0, att_w0 = g("rel_w0"), g("att_w0")   # (65, 64) each
    wi_cat = np.concatenate([rel_w0[:32], att_w0[:32]], axis=1)      # (32,128)
    wj_cat = np.concatenate([rel_w0[32:64], att_w0[32:64]], axis=1)  # (32,128)
    wd_cat = np.concatenate([rel_w0[64:65], att_w0[64:65]], axis=1)  # (1,128)
    b_l1 = np.concatenate([g("rel_b0"), g("att_b0")])                # (128,)

    # L2 fused: M-layout [att2(0:32) | pad(32:64) | ones(64) | rel2(65:97)]
    w1_aug = np.zeros((128, M2), np.float32)
    w1_aug[64:128, 0:32] = g("att_w1")
    w1_aug[0:64, 65:97] = g("rel_w1")
    b2_aug = np.zeros((M2,), np.float32)
    b2_aug[0:32] = g("att_b1")
    b2_aug[64] = 1.0
    b2_aug[65:97] = g("rel_b1")

    w_att2 = np.repeat(g("att_w2"), 33, axis=1)                      # (32,33)
    b3_att = np.zeros((M2,), np.float32)
    b3_att[OFS:OFS + 33] = float(g("att_b2").reshape(-1)[0])

    w2_g = np.zeros((33, 32), np.float32)
    w2_g[0] = g("rel_b2")
    w2_g[1:33] = g("rel_w2") + np.eye(32, dtype=np.float32)

    f = lambda a: np.ascontiguousarray(a, np.float32)
    pack32 = np.zeros((32, 712), np.float32)
    pack32[:, 0:128] = wi_cat
    pack32[:, 128:256] = wj_cat
    eye32 = np.eye(32, dtype=np.float32)
    folded = {"self_w1": g("self_w1") + eye32}
    for i, w in enumerate(["self_w0", "self_w1", "aff_w0", "aff_w1",
                           "aff_w2"]):
        pack32[:, 256 + 32 * i:288 + 32 * i] = folded.get(w, g(w))
    # aff3 is linear: fold aff_w2/aff_b2 into the out-head first layer
    pack32[:, 416:448] = g("aff_w2") @ g("out_w0")[0:32]
    pack32[:, 448:480] = g("out_w0")[32:64]
    pack32[:, 480:512] = g("out_w1")
    b_out0_f = g("out_b0") + g("aff_b2") @ g("out_w0")[0:32]
    for i, b in enumerate([b_s2, g("self_b0"), g("self_b1"), g("aff_b0"),
                           g("aff_b1"), g("aff_b2"), b_out0_f,
                           g("out_b1")]):
        pack32[:, 576 + i] = np.asarray(b, np.float32).reshape(-1)
    pack32[0, 584:712] = wd_cat[0]

    pack128 = np.zeros((128, 165), np.float32)
    pack128[:, 0:32] = w_s2
    pack128[:, 32] = b_l1
    pack128[:, 33:131] = w1_aug
    pack128[0:M2, 131] = b2_aug
    pack128[0:M2, 132] = b3_att
    pack128[64:97, 133:165] = w2_g

    return {
        "pack32": pack32, "pack128": pack128,
        "w_att2": bf(w_att2),
        "iden": np.eye(128, dtype=np.float32),
    }


def kernel(**inputs) -> np.ndarray:
    from concourse.bass_utils import run_bass_kernel_spmd

    nc = _get_program()
    weights = _prep_weights(inputs)
    s = np.asarray(inputs["s"], dtype=np.float32)
    in_maps = []
    for core in range(NCORES):
        m = dict(weights)
        m["s3d"] = np.ascontiguousarray(s[core * NP:(core + 1) * NP])
        in_maps.append(m)
    res = run_bass_kernel_spmd(nc, in_maps, list(range(NCORES)))
    out = np.concatenate([res.results[i]["out"] for i in range(NCORES)], axis=0)
    return out.astype(np.float32)


# revision 5
# speedup vs baseline: 1.7514x; 1.0049x over previous
"""Trainium2 Bass kernel for nn_Dynamics (GNN message passing).

Data-parallel over batch n=1024 across 8 NeuronCores (NP=128 rows each).
Channel-major on-chip layout: (channels, batch*obj); mixed precision
(f32r matmuls, bf16 on the rel2/attention product path).

Key structure: the rel tower's last Linear is lifted OUT of the per-pair
work via linearity of the attention-weighted sum:
    rel_dyn = sum_j (W2^T rel2 + rel2 + b) * att
            = (W2 + I)^T g  +  b * A,   g = sum_j rel2*att, A = sum_j att
so each 1024-pair batch row needs only 10 matmuls (L1 x6 accumulated
from x_i / x_j broadcasts and a streamed distance row, L2 x2 fused
[att2|ones|rel2], logit x2 with the scalar output replicated across 33
PSUM partitions via the M tile_position - free on the PE): PE 2133 ns.
Evacs are balanced across Act (relu2, exp), DVE (relu1, product, final
reduce) and gpsimd (halving adds, diagonal removal via stride-33 views),
each just under the PE budget; the PE stream is software-pipelined
across rows (l3(r-3), l1(r), l2(r-1)) so matmuls never wait on same-row
evacuations, and all phases share one SBUF/PSUM pool so no inter-phase
drain barriers are emitted. The affector's final Linear is folded into
the output head host-side. TimelineSim: ~343 us vs 589 us baseline.
"""

import numpy as np
import ml_dtypes
from contextlib import ExitStack

import concourse.bass as bass
import concourse.mybir as mybir
import concourse.tile as tile
from concourse import bacc

F32 = mybir.dt.float32
F32R = mybir.dt.float32r
BF16 = mybir.dt.bfloat16
AF = mybir.ActivationFunctionType
ALU = mybir.AluOpType
AX = mybir.AxisListType

N = 1024
NOBJ = 32
CL = 32
NCORES = 8
NP = N // NCORES          # 128 batch rows per core
NO = NP * NOBJ            # 4096 objects per core
BLK = 512                 # pairs per PSUM bank (fp32)
NCHUNK = 32               # dist stream chunks of 4 batch rows
CHUNK_N = NP // NCHUNK    # 8

# M-layout of the fused L2 output (98 rows):
#   0:32   att2 (att tower hidden)
#   32:64  zero pad (so the ones/rel block lands at partition 64)
#   64     ones row (bias 1.0, zero weights) -> A accumulator
#   65:97  rel2 (rel tower hidden)
M2 = 98
OFS = 64                  # partition offset of [ones; rel2] block


def _build_program():
    nc = bacc.Bacc("TRN2", target_bir_lowering=False, debug=False)

    def din(name, shape, dt=F32):
        return nc.dram_tensor(name, list(shape), dt, kind="ExternalInput").ap()

    s3d = din("s3d", (NP, NOBJ, 16))
    sT = din("sT", (16, NP * NOBJ))    # host-transposed state, channel-major
    # pack32: [wi(0:128) | wj(128:256) | self0 self1 aff0 aff1 aff2 out0a
    #          out0b out1 (256:576 in 32-col strips) | 8 biases (576:584) |
    #          wd on row 0 (584:712)]
    pack32 = din("pack32", (32, 712))
    # pack128: [w_s2(0:32) | b_l1(32) | w1_aug(33:131) | b2_aug(131) |
    #           b3_att(132) | w2_g rows 64:97 (133:165)]
    pack128 = din("pack128", (128, 165))
    w_att2 = din("w_att2", (32, 33), BF16)  # att_w2 replicated 33x
    iden = din("iden", (128, 128))

    out_d = nc.dram_tensor("out", [NP, NOBJ, CL], F32, kind="ExternalOutput").ap()

    with tile.TileContext(nc) as tc, ExitStack() as ctx:
        const = ctx.enter_context(tc.tile_pool(name="const", bufs=1))

        def load_const(ap_in, shape, dt=F32):
            nm = f"t_{ap_in.name}"
            t = const.tile(list(shape), dt, name=nm, tag=nm)
            nc.gpsimd.dma_start(out=t, in_=ap_in)
            return t

        def load_const_at(ap_in, shape, ofs, dt=F32):
            nm = f"t_{ap_in.name}"
            t = const.tile([ofs + shape[0]] + list(shape[1:]), dt,
                           name=nm, tag=nm)
            nc.gpsimd.dma_start(out=t[ofs:], in_=ap_in)
            return t[ofs:]

        def load_const_r(ap_in, shape, ofs=0):
            # f32 DMA load + rounded copy to an f32r tile at partition `ofs`
            # (matmul fmap/weight must share the SBUF base partition)
            nm = f"t_{ap_in.name}"
            tf = const.tile([ofs + shape[0]] + list(shape[1:]), F32,
                            name=nm + "_f", tag=nm + "_f")
            nc.gpsimd.dma_start(out=tf[ofs:], in_=ap_in)
            tr = const.tile([ofs + shape[0]] + list(shape[1:]), F32R,
                            name=nm, tag=nm)
            nc.vector.tensor_copy(tr[ofs:], tf[ofs:])
            return tr[ofs:]

        # critical-path loads first: identity + state, spread over the
        # three DMA-capable queues (SP, Act, Pool); packed consts after.
        t_iden = const.tile([128, 128], F32, name="t_iden", tag="t_iden")
        s_pad = const.tile([NP, NOBJ, 32], F32)
        nc.gpsimd.memset(s_pad[:, :, 16:32], 0.0)
        nc.sync.dma_start(out=t_iden, in_=iden)

        nc.sync.dma_start(out=s_pad[0:48, :, 0:16], in_=s3d[0:48])
        nc.scalar.dma_start(out=s_pad[48:96, :, 0:16], in_=s3d[48:96])
        nc.gpsimd.dma_start(out=s_pad[96:128, :, 0:16], in_=s3d[96:128])

        p32f = const.tile([32, 712], F32, name="p32f", tag="p32f")
        nc.scalar.dma_start(out=p32f, in_=pack32)
        p128f = const.tile([128, 165], F32, name="p128f", tag="p128f")
        nc.gpsimd.dma_start(out=p128f, in_=pack128)
        t_wat2 = const.tile([32, 33], BF16, name="t_wat2", tag="t_wat2")
        nc.scalar.dma_start(out=t_wat2, in_=w_att2)

        p32r = const.tile([32, 712], F32R, name="p32r", tag="p32r")
        p128r = const.tile([128, 165], F32R, name="p128r", tag="p128r")
        nc.vector.tensor_copy(p128r, p128f)

        t_wi = p32r[:, 0:128]
        t_wj = p32r[:, 128:256]
        t_wself0 = p32r[:, 256:288]
        t_wself1 = p32r[:, 288:320]
        t_waff0 = p32r[:, 320:352]
        t_waff1 = p32r[:, 352:384]
        t_waff2 = p32r[:, 384:416]
        t_wout0a = p32r[:, 416:448]
        t_wout0b = p32r[:, 448:480]
        t_wout1 = p32r[:, 480:512]
        t_bs2 = p32f[:, 576:577]
        t_bself0 = p32f[:, 577:578]
        t_bself1 = p32f[:, 578:579]
        t_baff0 = p32f[:, 579:580]
        t_baff1 = p32f[:, 580:581]
        t_baff2 = p32f[:, 581:582]
        t_bout0 = p32f[:, 582:583]
        t_bout1 = p32f[:, 583:584]
        t_wd = p32r[0:1, 584:712]
        t_ws2 = p128r[0:16, 0:32]
        t_bl1 = p128f[:, 32:33]
        t_w1a = p128r[:, 33:131]
        t_b2a = p128f[0:M2, 131:132]
        t_b3a = p128f[:, 132:133]
        t_w2g = p128r[64:97, 133:165]
        t_iden = const.tile([128, 128], F32, name="t_iden", tag="t_iden")
        nc.sync.dma_start(out=t_iden, in_=iden)


        # persistent activations
        s2 = const.tile([32, NO], F32R)
        dist_tr = const.tile([NP, NOBJ * NOBJ], F32R)    # (128, 1024)
        g_cm = const.tile([M2, NO], F32R)        # rows 64:97 = [A; g]

        # --- prologue: s2 build, dist ---

        wsb_cm = tc.tile_pool(name="wsb", bufs=1)
        wsb = wsb_cm.__enter__()
        wps_cm = tc.tile_pool(name="wps", bufs=2, space="PSUM")
        wps = wps_cm.__enter__()

        # warm the PE p-state during DMA waits: ~40 junk matmuls on the
        # identity keep the ramp counter running so the loop's first rows
        # run at full clock (results unread).
        for wu in range(28):
            pwu = wps.tile([128, 128], F32, tag="p1", name="pwu")
            nc.tensor.matmul(pwu, t_iden, t_iden, start=True, stop=True)

        # s2 = concat(pos, enc[2:]) from the host-transposed state: one
        # f32r conversion (Act, two halves) + 8 K=16 matmuls, no transposes
        t_sTf = wsb.tile([16, NO], F32, tag="tb", bufs=4, name="t_sTf")
        nc.sync.dma_start(out=t_sTf, in_=sT)
        sTr = wsb.tile([16, NO], F32R, tag="tb", bufs=4, name="sTr")
        for hh in range(2):
            nc.scalar.activation(sTr[:, hh * 2048:(hh + 1) * 2048],
                                 t_sTf[:, hh * 2048:(hh + 1) * 2048], AF.Copy)
        for w2 in range(4):
            ps = wps.tile([32, 1024], F32, tag="p1", name="ps_enc")
            for h in range(2):
                sl = slice(w2 * 1024 + h * BLK, w2 * 1024 + (h + 1) * BLK)
                nc.tensor.matmul(ps[:, h * BLK:(h + 1) * BLK], t_ws2,
                                 sTr[:, sl], start=True, stop=True)
            nc.scalar.activation(s2[:, w2 * 1024:(w2 + 1) * 1024], ps,
                                 AF.Identity, bias=t_bs2)

        # pairwise squared distances (n on partitions), fp32
        posT = s_pad.rearrange("p a b -> p b a")[:, 0:2]   # (128, 2, 32)
        diff = wsb.tile([NP, 2, NOBJ, NOBJ], F32, tag="tb", bufs=4, name="diff")
        nc.vector.tensor_tensor(
            out=diff,
            in0=posT.unsqueeze(3).broadcast_to([NP, 2, NOBJ, NOBJ]),
            in1=posT.unsqueeze(2).broadcast_to([NP, 2, NOBJ, NOBJ]),
            op=ALU.subtract)
        sq = wsb.tile([NP, 2, NOBJ, NOBJ], F32, tag="tb", bufs=4, name="sq")
        nc.vector.tensor_tensor(out=sq, in0=diff, in1=diff, op=ALU.mult)
        nc.vector.tensor_tensor(
            out=dist_tr.rearrange("p (a b) -> p a b", b=NOBJ),
            in0=sq[:, 0], in1=sq[:, 1], op=ALU.add)
        nc.vector.tensor_copy(p32r[:, 0:576], p32f[:, 0:576])
        nc.vector.tensor_copy(p32r[0:1, 584:712], p32f[0:1, 584:712])

        # --- pair loop: one batch row (1024 pairs) per iteration, SW-pipelined ---

        P1 = {}      # rolling per-stage tiles keyed by row
        PAIR1 = {}
        P2 = {}
        R2 = {}
        P3 = {}
        ATTW = {}
        PRODT = {}
        PRODH = {}
        DPROD = {}
        DISTF = {}

        def prefetch(c):
            if c < NCHUNK and c not in DISTF:
                df = wsb.tile([1, CHUNK_N * 1024], F32R, tag="dist_f",
                                bufs=2, name="df")
                nc.sync.dma_start(
                    out=df, in_=dist_tr[CHUNK_N * c: CHUNK_N * (c + 1)])
                DISTF[c] = df

        def stage_dma(r):
            # prefetch the NEXT dist chunk while this one is consumed
            if r % CHUNK_N == 0:
                prefetch(r // CHUNK_N + 1)

        def stage_l1(r):
            # L1: p1 = Wi^T x_i + Wj^T x_j + wd^T dist  over (128, 1024)
            p1 = wps.tile([128, 1024], F32, tag="p1", name="p1").rearrange(
                "p (a b c) -> p a b c", b=16, c=NOBJ)
            df = DISTF[r // CHUNK_N]
            nl = r % CHUNK_N
            for ih in range(2):
                xi = s2[:, r * 32 + ih * 16: r * 32 + ih * 16 + 16]
                xi = xi.unsqueeze(2).broadcast_to([32, 16, NOBJ])
                xj = s2[:, r * 32: r * 32 + 32]
                xj = xj.unsqueeze(1).broadcast_to([32, 16, NOBJ])
                drow = df[0:1, nl * 1024 + ih * BLK: nl * 1024 + (ih + 1) * BLK]
                nc.tensor.matmul(p1[:, ih], t_wi, xi, start=True, stop=False)
                nc.tensor.matmul(p1[:, ih], t_wj, xj, start=False, stop=False)
                nc.tensor.matmul(
                    p1[:, ih].rearrange("p a b -> p (a b)"),
                    t_wd, drow, start=False, stop=True)
            P1[r] = p1

        def stage_relu1(r):
            # DVE evac: pair1 = relu(p1 + b)
            pair1 = wsb.tile([128, 1024], F32R, tag="pair1", bufs=2, name="pair1")
            nc.vector.tensor_scalar(
                out=pair1, in0=P1[r].rearrange("p a b c -> p (a b c)"),
                scalar1=t_bl1, scalar2=0.0, op0=ALU.add, op1=ALU.max)
            del P1[r]
            PAIR1[r] = pair1

        def stage_l2(r):
            p2 = wps.tile([M2, 2, BLK], F32, tag="p2", bufs=1, name="p2")
            for ih in range(2):
                nc.tensor.matmul(p2[:, ih], t_w1a,
                                 PAIR1[r][:, ih * BLK:(ih + 1) * BLK],
                                 start=True, stop=True)
            del PAIR1[r]
            P2[r] = p2

        def stage_relu2(r):
            # Act evac, single (98, 1024) op
            r2 = wsb.tile([M2, 1024], BF16, tag="r2", bufs=3, name="r2")
            nc.scalar.activation(
                r2, P2[r].rearrange("p a b -> p (a b)"), AF.Relu, bias=t_b2a)
            del P2[r]
            R2[r] = r2

        def stage_l3(r):
            # logit matmul: M=33 replicated output at partitions 64:97
            p3 = wps.tile([M2, 1024], F32, tag="p3", bufs=1, name="p3")
            for ih in range(2):
                nc.tensor.matmul(
                    p3[OFS:OFS + 33, ih * BLK:(ih + 1) * BLK], t_wat2,
                    R2[r][0:32, ih * BLK:(ih + 1) * BLK],
                    start=True, stop=True, tile_position=(0, OFS))
            P3[r] = p3

        def stage_exp(r):
            attw = wsb.tile([M2, 1024], BF16, tag="attw", bufs=3, name="attw")
            nc.scalar.activation(attw[OFS:OFS + 33], P3[r][OFS:OFS + 33],
                                 AF.Exp, bias=t_b3a[OFS:OFS + 33])
            del P3[r]
            ATTW[r] = attw

        def stage_prod(r):
            prodt = wsb.tile([M2, 1024], BF16, tag="prodt", bufs=2, name="prodt")
            nc.vector.tensor_tensor(
                out=prodt[OFS:OFS + 33], in0=R2[r][OFS:OFS + 33],
                in1=ATTW[r][OFS:OFS + 33], op=ALU.mult)
            PRODT[r] = prodt

        def stage_diagmult(r):
            dprod_t = wsb.tile([OFS + 33, 32], F32R, tag="dprod", bufs=3,
                                name="dprod")
            nc.gpsimd.tensor_tensor(
                out=dprod_t[OFS:OFS + 33], in0=R2[r][OFS:OFS + 33, 0:1024:33],
                in1=ATTW[r][OFS:OFS + 33, 0:1024:33], op=ALU.mult)
            DPROD[r] = dprod_t

        def stage_halve(r):
            # two binary-tree halving adds over j on gpsimd
            p4 = PRODT[r][OFS:OFS + 33].rearrange("p (a b) -> p a b", b=NOBJ)
            ph_t = wsb.tile([OFS + 33, 32, 16], F32, tag="prodh1", bufs=2,
                             name="ph1")
            nc.gpsimd.tensor_tensor(
                out=ph_t[OFS:OFS + 33], in0=p4[:, :, 0:16], in1=p4[:, :, 16:32],
                op=ALU.add)
            ph2_t = wsb.tile([OFS + 33, 32, 8], F32, tag="prodh2", bufs=2,
                              name="ph2")
            nc.gpsimd.tensor_tensor(
                out=ph2_t[OFS:OFS + 33], in0=ph_t[OFS:OFS + 33, :, 0:8],
                in1=ph_t[OFS:OFS + 33, :, 8:16], op=ALU.add)
            del PRODT[r]
            PRODH[r] = ph2_t

        def stage_reduce(r):
            # g[:, 32 cols of row r] = sum_j prod; then subtract diagonal
            gsl = g_cm[OFS:OFS + 33, r * 32:(r + 1) * 32]
            with nc.allow_low_precision(reason="f32r g accumulation (8 terms)"):
                nc.vector.tensor_reduce(
                    gsl, PRODH[r][OFS:OFS + 33], AX.X, ALU.add)
            del PRODH[r]

        def stage_diagsub(r):
            gsl = g_cm[OFS:OFS + 33, r * 32:(r + 1) * 32]
            nc.gpsimd.tensor_tensor(out=gsl, in0=gsl.bitcast(F32),
                                    in1=DPROD[r][OFS:OFS + 33].bitcast(F32),
                                    op=ALU.subtract)
            del DPROD[r]

        # software pipeline: stages of row r execute in iteration r+lag.
        # per-iteration PE order is l3(r-3), l1(r), l2(r-1) so each matmul's
        # producer evac finished at least one iteration earlier.
        STAGES = [
            (0, stage_dma),
            (3, stage_l3),
            (0, stage_l1),
            (1, stage_relu1),
            (1, stage_l2),
            (2, stage_relu2),
            (3, stage_exp),
            (4, stage_prod),
            (4, stage_diagmult),
            (5, stage_halve),
            (6, stage_reduce),
            (6, stage_diagsub),
        ]
        prefetch(0)
        DEPTH = 7
        for it in range(NP + DEPTH - 1):
            for lag, fn in STAGES:
                r = it - lag
                if 0 <= r < NP:
                    fn(r)

        # --- tail: rel_dyn from g; dyn; affector (aff3 folded into the
        # out-head weights); 2-block (32, 1024) psum evacs ---
        W = 1024
        h1 = wsb.tile([32, NO], F32R, tag="tb", bufs=4, name="h1")
        selfd = wsb.tile([32, NO], F32, tag="tb", bufs=4, name="selfd")
        dyn = wsb.tile([32, NO], F32R, tag="tb", bufs=4, name="dyn")
        aff1 = wsb.tile([32, NO], F32R, tag="tb", bufs=4, name="aff1")
        aff2 = wsb.tile([32, NO], F32R, tag="tb", bufs=4, name="aff2")
        o1 = wsb.tile([32, NO], F32R, tag="tb", bufs=4, name="o1")
        res_cm = wsb.tile([32, NO], F32, tag="tb", bufs=4, name="res")

        def wsl(w):
            return slice(w * W, (w + 1) * W)

        for w in range(NO // W):
            ps = wps.tile([32, W], F32, tag=["p1", "p2", "p3", "p1"][w], bufs=[2, 1, 1, 2][w], name="psa")
            for h in range(2):
                hs = slice(w * W + h * BLK, w * W + (h + 1) * BLK)
                nc.tensor.matmul(ps[:, h * BLK:(h + 1) * BLK], t_wself0,
                                 s2[:, hs], start=True, stop=True)
            nc.vector.tensor_scalar(
                out=h1[:, wsl(w)], in0=ps, scalar1=t_bself0, scalar2=0.0,
                op0=ALU.add, op1=ALU.max)
        for w in range(NO // W):
            ps = wps.tile([32, W], F32, tag=["p1", "p2", "p3", "p1"][w], bufs=[2, 1, 1, 2][w], name="psb")
            for h in range(2):
                hs = slice(w * W + h * BLK, w * W + (h + 1) * BLK)
                nc.tensor.matmul(ps[:, h * BLK:(h + 1) * BLK], t_wself1,
                                 h1[:, hs], start=True, stop=True)
            nc.scalar.activation(selfd[:, wsl(w)], ps, AF.Identity,
                                 bias=t_bself1)
        for w in range(NO // W):
            ps = wps.tile([32, W], F32, tag=["p1", "p2", "p3", "p1"][w], bufs=[2, 1, 1, 2][w], name="psc")
            for h in range(2):
                hs = slice(w * W + h * BLK, w * W + (h + 1) * BLK)
                nc.tensor.matmul(ps[:, h * BLK:(h + 1) * BLK], t_w2g,
                                 g_cm[OFS:OFS + 33, hs],
                                 start=True, stop=True,
                                 tile_position=(OFS, 0))
            nc.vector.scalar_tensor_tensor(
                out=dyn[:, wsl(w)], in0=ps, scalar=0.0,
                in1=selfd[:, wsl(w)], op0=ALU.add, op1=ALU.add)
        for w in range(NO // W):
            ps = wps.tile([32, W], F32, tag=["p1", "p2", "p3", "p1"][w], bufs=[2, 1, 1, 2][w], name="psd")
            for h in range(2):
                hs = slice(w * W + h * BLK, w * W + (h + 1) * BLK)
                nc.tensor.matmul(ps[:, h * BLK:(h + 1) * BLK], t_waff0,
                                 dyn[:, hs], start=True, stop=True)
            nc.scalar.activation(aff1[:, wsl(w)], ps, AF.Tanh, bias=t_baff0)

        for w in range(NO // W):
            ps = wps.tile([32, W], F32, tag=["p1", "p2", "p3", "p1"][w], bufs=[2, 1, 1, 2][w], name="pse")
            for h in range(2):
                hs = slice(w * W + h * BLK, w * W + (h + 1) * BLK)
                nc.tensor.matmul(ps[:, h * BLK:(h + 1) * BLK], t_waff1,
                                 aff1[:, hs], start=True, stop=True)
            tmp = wsb.tile([32, W], F32R, tag="afftmp", bufs=2, name="tmp")
            nc.scalar.activation(tmp, ps, AF.Tanh, bias=t_baff1)
            nc.vector.tensor_tensor(out=aff2[:, wsl(w)],
                                    in0=tmp.bitcast(F32),
                                    in1=aff1[:, wsl(w)].bitcast(F32),
                                    op=ALU.add)
        for w in range(NO // W):
            ps = wps.tile([32, W], F32, tag=["p1", "p2", "p3", "p1"][w], bufs=[2, 1, 1, 2][w], name="psf")
            for h in range(2):
                hs = slice(w * W + h * BLK, w * W + (h + 1) * BLK)
                nc.tensor.matmul(ps[:, h * BLK:(h + 1) * BLK], t_wout0a,
                                 aff2[:, hs], start=True, stop=False)
                nc.tensor.matmul(ps[:, h * BLK:(h + 1) * BLK], t_wout0b,
                                 s2[:, hs], start=False, stop=True)
            nc.scalar.activation(o1[:, wsl(w)], ps, AF.Tanh, bias=t_bout0)
        for w in range(NO // W):
            ps = wps.tile([32, W], F32, tag=["p1", "p2", "p3", "p1"][w], bufs=[2, 1, 1, 2][w], name="psg")
            for h in range(2):
                hs = slice(w * W + h * BLK, w * W + (h + 1) * BLK)
                nc.tensor.matmul(ps[:, h * BLK:(h + 1) * BLK], t_wout1,
                                 o1[:, hs], start=True, stop=True)
            nc.vector.scalar_tensor_tensor(
                out=res_cm[:, wsl(w)], in0=ps, scalar=t_bout1,
                in1=o1[:, wsl(w)].bitcast(F32), op0=ALU.add, op1=ALU.add)

        # transpose back to (n, o, c) and store, 4 blocks per group
        for gq in range(2):
            pst2 = wps.tile([128, 512], F32, tag="p1", name="pst2")
            for j in range(4):
                b = 4 * gq + j
                q, c = b % 4, b // 4
                for k in range(4):
                    nc.tensor.transpose(
                        pst2[:, 128 * j + 32 * k:128 * j + 32 * (k + 1)],
                        res_cm[:, BLK * b + 128 * k: BLK * b + 128 * (k + 1)],
                        t_iden[0:32, 0:32])
            o_sb = wsb.tile([128, 512], F32, tag="o_sb", bufs=2, name="o_sb")
            nc.scalar.activation(o_sb, pst2, AF.Copy)
            nc.sync.dma_start(
                out=out_d.rearrange("a o c -> (a o) c")[
                    2048 * gq:2048 * (gq + 1)].rearrange(
                    "(j k p) c -> p j k c", j=4, k=4),
                in_=o_sb.rearrange("p (j k c) -> p j k c", j=4, c=32))
        wps_cm.__exit__(None, None, None)
        wsb_cm.__exit__(None, None, None)

    nc.compile()
    return nc


_PROG = None


def _get_program():
    global _PROG
    if _PROG is None:
        _PROG = _build_program()
    return _PROG


def _prep_weights(inp):
    g = lambda k: np.asarray(inp[k], dtype=np.float32)
    bf = lambda a: np.ascontiguousarray(a, dtype=ml_dtypes.bfloat16)
    col = lambda v: np.ascontiguousarray(
        np.asarray(v, np.float32).reshape(-1, 1), dtype=np.float32)

    w_s2 = g("state_enc_w").copy()      # (16, 32)
    b_s2 = g("state_enc_b").copy()
    w_s2[:, 0:2] = 0.0
    w_s2[0, 0] = 1.0
    w_s2[1, 1] = 1.0
    b_s2[0:2] = 0.0

    rel_w0, att_w0 = g("rel_w0"), g("att_w0")   # (65, 64) each
    wi_cat = np.concatenate([rel_w0[:32], att_w0[:32]], axis=1)      # (32,128)
    wj_cat = np.concatenate([rel_w0[32:64], att_w0[32:64]], axis=1)  # (32,128)
    wd_cat = np.concatenate([rel_w0[64:65], att_w0[64:65]], axis=1)  # (1,128)
    b_l1 = np.concatenate([g("rel_b0"), g("att_b0")])                # (128,)

    # L2 fused: M-layout [att2(0:32) | pad(32:64) | ones(64) | rel2(65:97)]
    w1_aug = np.zeros((128, M2), np.float32)
    w1_aug[64:128, 0:32] = g("att_w1")
    w1_aug[0:64, 65:97] = g("rel_w1")
    b2_aug = np.zeros((M2,), np.float32)
    b2_aug[0:32] = g("att_b1")
    b2_aug[64] = 1.0
    b2_aug[65:97] = g("rel_b1")

    w_att2 = np.repeat(g("att_w2"), 33, axis=1)                      # (32,33)
    b3_att = np.zeros((M2,), np.float32)
    b3_att[OFS:OFS + 33] = float(g("att_b2").reshape(-1)[0])

    w2_g = np.zeros((33, 32), np.float32)
    w2_g[0] = g("rel_b2")
    w2_g[1:33] = g("rel_w2") + np.eye(32, dtype=np.float32)

    f = lambda a: np.ascontiguousarray(a, np.float32)
    pack32 = np.zeros((32, 712), np.float32)
    pack32[:, 0:128] = wi_cat
    pack32[:, 128:256] = wj_cat
    eye32 = np.eye(32, dtype=np.float32)
    folded = {"self_w1": g("self_w1") + eye32}
    for i, w in enumerate(["self_w0", "self_w1", "aff_w0", "aff_w1",
                           "aff_w2"]):
        pack32[:, 256 + 32 * i:288 + 32 * i] = folded.get(w, g(w))
    # aff3 is linear: fold aff_w2/aff_b2 into the out-head first layer
    pack32[:, 416:448] = g("aff_w2") @ g("out_w0")[0:32]
    pack32[:, 448:480] = g("out_w0")[32:64]
    pack32[:, 480:512] = g("out_w1")
    b_out0_f = g("out_b0") + g("aff_b2") @ g("out_w0")[0:32]
    for i, b in enumerate([b_s2, g("self_b0"), g("self_b1"), g("aff_b0"),
                           g("aff_b1"), g("aff_b2"), b_out0_f,
                           g("out_b1")]):
        pack32[:, 576 + i] = np.asarray(b, np.float32).reshape(-1)
    pack32[0, 584:712] = wd_cat[0]

    pack128 = np.zeros((128, 165), np.float32)
    pack128[0:16, 0:32] = w_s2
    pack128[:, 32] = b_l1
    pack128[:, 33:131] = w1_aug
    pack128[0:M2, 131] = b2_aug
    pack128[0:M2, 132] = b3_att
    pack128[64:97, 133:165] = w2_g

    return {
        "pack32": pack32, "pack128": pack128,
        "w_att2": bf(w_att2),
        "iden": np.eye(128, dtype=np.float32),
    }


def kernel(**inputs) -> np.ndarray:
    from concourse.bass_utils import run_bass_kernel_spmd

    nc = _get_program()
    weights = _prep_weights(inputs)
    s = np.asarray(inputs["s"], dtype=np.float32)
    in_maps = []
    for core in range(NCORES):
        m = dict(weights)
        sc = s[core * NP:(core + 1) * NP]
        m["s3d"] = np.ascontiguousarray(sc)
        m["sT"] = np.ascontiguousarray(sc.reshape(NP * NOBJ, 16).T)
        in_maps.append(m)
    res = run_bass_kernel_spmd(nc, in_maps, list(range(NCORES)))
    out = np.concatenate([res.results[i]["out"] for i in range(NCORES)], axis=0)
    return out.astype(np.float32)
